# revision 22
# baseline (speedup 1.0000x reference)
"""DiT block (adaLN) Trainium2 kernel, 8-core SPMD, no collectives.

Sharding: core c handles batch b = c//2 and query-token half c%2 (1024 q
tokens).  Each core computes K/V for all 2048 tokens of its batch (the
only duplicated work), so cores never communicate.  The host permutes
each core's token columns so its own 1024 tokens come first (softmax is
invariant to key order), and transposes x to feature-major [D, L] so the
device never transposes anything.

On-device layout is feature-major everywhere: activations live as
[128 partitions, d-chunk, tokens].  LayerNorm stats (per-token = free
dim) are computed with ones-vector matmuls on the tensor engine and
broadcast back across partitions with stride-0 DMA.  All GEMM operands
are bf16 (fp32 PSUM accumulation); the residual stream, softmax and LN
statistics stay fp32.

Perf structure (vs the first working version):
- All weights are staged into SBUF with large multi-dim DMAs (one per
  output-feature chunk) instead of one DMA per 128x128 tile; the small
  DMAs serialized on the sync sequencer (~600ns each, ~1150 of them).
- Attention is software-pipelined at key-chunk granularity: the PE does
  QK(g+1) and AV(g-1) while the scalar engine does exp(g), so the PE
  never idles long enough for the HAM clock gate to re-throttle.
- Attention output is written straight into SBUF (partition-offset
  writes per head) instead of bouncing through DRAM.
"""

import os
import sys
from contextlib import ExitStack

os.environ.setdefault("MYCRO_LOCAL_CACHE", "1")
for _p in ("/opt/trn_rl_repo", "/root/.axon_site/_ro/trn_rl_repo"):
    if os.path.isdir(_p) and _p not in sys.path:
        sys.path.insert(0, _p)

import ml_dtypes
import numpy as np

import concourse.bass as bass
import concourse.tile as tile
from concourse import bacc, mybir
from concourse.bass_utils import run_bass_kernel_spmd

B, L, D, H, HD, MLPD = 4, 2048, 1024, 16, 64, 4096
NCORES = 8
LOWN = L // 2          # own query tokens per core
DC = D // 128          # 8 chunks of the model dim
MC = MLPD // 128       # 32 chunks of the mlp dim
LT = 512               # token tile for matmul free dim
NLT_OWN = LOWN // LT   # 2 token tiles (queries)
NKC = L // 128         # 16 key chunks

f32 = mybir.dt.float32
bf16 = mybir.dt.bfloat16
AF = mybir.ActivationFunctionType
ALU = mybir.AluOpType
BF = ml_dtypes.bfloat16


def _bcast_rows(nc, pool, row_ap, nrows, ncols, tag, dtype=None, bufs=2):
    """SBUF [nrows, ncols] tile = row_ap ([1, ncols] SBUF) broadcast
    across partitions, on the otherwise-idle GpSimd engine."""
    dtype = dtype or mybir.dt.float32
    out = pool.tile([nrows, ncols], dtype, tag=tag, bufs=bufs, name=tag)
    nc.gpsimd.partition_broadcast(out, row_ap)
    return out


def build_program():
    # Bacc (not plain Bass): its compile() pass legalizes multi-semaphore
    # waits (event semaphores, nop fusion) that walrus can't encode raw.
    nc = bacc.Bacc()

    def _in(name, shape, dtype):
        return nc.declare_dram_parameter(name, shape, dtype, False)[:]

    xfm = _in("xfm", [D, LOWN], f32)
    xoth = _in("xoth", [D, LOWN], bf16)
    temb = _in("temb", [128, DC], f32)
    wqkv = _in("wqkv", [D, 3 * D], bf16)
    bq = _in("bq", [128, DC], f32)     # pre-scaled by 1/8
    bk = _in("bk", [128, DC], f32)
    bv = _in("bv", [1, D], f32)
    wproj = _in("wproj", [D, D], bf16)
    bproj = _in("bproj", [128, DC], f32)
    w1 = _in("w1", [D, MLPD], bf16)
    b1 = _in("b1", [128, MC], f32)
    w2 = _in("w2", [MLPD, D], bf16)
    b2 = _in("b2", [128, DC], f32)
    wt = _in("wt", [D, 6 * D], bf16)
    bt = _in("bt", [128, 48], f32)
    out = nc.declare_dram_parameter("out_fm", [D, LOWN], f32, True)[:]

    with tile.TileContext(nc) as tc:
        _emit_kernel(tc, xfm, xoth, temb, wqkv, bq, bk, bv, wproj, bproj,
                     w1, b1, w2, b2, wt, bt, out)
    nc.finalize()  # runs Bacc.compile(): reg alloc + sync legalization
    return nc


def _emit_kernel(tc, xfm, xoth, temb, wqkv, bq, bk, bv, wproj, bproj, w1, b1,
                 w2, b2, wt, bt, out):
    nc = tc.nc

    # feature-major views of the weight matrices: [128, in-chunk, out-col]
    wt_r = wt.rearrange("(c p) n -> p c n", p=128)
    wqkv_r = wqkv.rearrange("(c p) n -> p c n", p=128)
    wproj_r = wproj.rearrange("(c p) n -> p c n", p=128)
    w1_r = w1.rearrange("(c p) n -> p c n", p=128)
    w2_r = w2.rearrange("(c p) n -> p c n", p=128)

    # ---- persistent constants / host-prepped vectors (freed last) ----
    ones_bf, fr_ones_bf = tc.tile([128, 1], bf16, name="ones_bf")
    nc.vector.memset(ones_bf, 1.0)
    eps_tile, fr_eps = tc.tile([1, 1], f32, name="eps_tile")
    nc.vector.memset(eps_tile, 1e-5)

    bias_sb = {}
    bias_frees = []
    for name, ap, w in (("bq", bq, DC), ("bk", bk, DC), ("bproj", bproj, DC),
                        ("b1", b1, MC), ("b2", b2, DC), ("bt", bt, 48),
                        ("temb", temb, DC)):
        t, fr = tc.tile([128, w], f32, name=f"sb_{name}")
        nc.sync.dma_start(out=t, in_=ap)
        bias_sb[name] = t
        bias_frees.append(fr)
    bv_bc, fr_bv = tc.tile([128, D], f32, name="bv_bc")
    nc.sync.dma_start(
        out=bv_bc,
        in_=bass.AP(tensor=bv.tensor, offset=bv.offset,
                    ap=[[0, 128]] + [list(x) for x in bv.ap[1:]]))

    # modulation vectors (computed in phase 0, consumed later)
    tp, fr_tp = tc.tile([128, 48], f32, name="tp")
    s_msa, fr_s1 = tc.tile([128, DC], f32, name="s_msa")
    s_mlp, fr_s2 = tc.tile([128, DC], f32, name="s_mlp")
    gmbp, fr_g1 = tc.tile([128, DC], f32, name="gmbp")
    gmb2, fr_g2 = tc.tile([128, DC], f32, name="gmb2")
    shift_msa = tp[:, 0:8]
    gate_msa = tp[:, 16:24]
    shift_mlp = tp[:, 24:32]
    gate_mlp = tp[:, 40:48]

    # ---- big persistent activations ----
    x_own, fr_x_own = tc.tile([128, DC, LOWN], f32, name="x_own")
    k_sb, fr_k = tc.tile([128, DC, L], bf16, name="k_sb")
    # v_aug: [token-part, token-chunk, head, 65]; col 64 holds ones so the
    # AV matmul also produces the softmax denominator.
    v_aug, fr_v = tc.tile([128, NKC, H, HD + 1], bf16, name="v_aug")
    q_sb, fr_q = tc.tile([128, DC, LOWN], bf16, name="q_sb")
    xmod, fr_xmod = tc.tile([128, DC, L], bf16, name="xmod")

    xr = xfm.rearrange("(c p) t -> p c t", p=128)
    nc.sync.dma_start(out=x_own, in_=xr)
    # other token half arrives pre-cast to bf16 and lands directly in xmod;
    # LN1 then runs in place on it.
    xor_ = xoth.rearrange("(c p) t -> p c t", p=128)
    nc.sync.dma_start(out=xmod[:, :, LOWN:], in_=xor_)

    # ---- LayerNorm helpers: bf16, in place on a [128, DC, LT] bf16 tile ----
    def ln_stats(sbp, psp, xm_view):
        """Stats over the (pre-modulation) bf16 tile; returns broadcast
        tiles (a_bc=rstd, m_bc=mu*rstd) in bf16."""
        ps_s = psp.tile([1, LT], f32, tag="st_s", bufs=2, name="ps_s")
        ps_q = psp.tile([1, LT], f32, tag="st_q", bufs=2, name="ps_q")
        for dc in range(DC):
            xs = xm_view[:, dc, :]
            nc.tensor.matmul(ps_s, ones_bf, xs,
                             start=(dc == 0), stop=(dc == DC - 1))
            sq = sbp.tile([128, LT], bf16, tag="sq", bufs=2, name="sq")
            nc.vector.tensor_tensor(sq, xs, xs, ALU.mult)
            nc.tensor.matmul(ps_q, ones_bf, sq,
                             start=(dc == 0), stop=(dc == DC - 1))
        mean = sbp.tile([1, LT], f32, tag="ln_mean", bufs=2, name="mean")
        var = sbp.tile([1, LT], f32, tag="ln_var", bufs=2, name="var")
        msq = sbp.tile([1, LT], f32, tag="ln_msq", bufs=2, name="msq")
        nc.vector.tensor_scalar_mul(mean, ps_s, 1.0 / D)
        nc.vector.tensor_scalar_mul(var, ps_q, 1.0 / D)
        nc.vector.tensor_tensor(msq, mean, mean, ALU.mult)
        nc.vector.tensor_tensor(var, var, msq, ALU.subtract)
        # rstd = (var+eps)^-0.5 as Exp(-0.5*Ln(var+eps)): both functions sit
        # in one scalar table set, and it avoids the slow DVE reciprocal.
        lnv = sbp.tile([1, LT], f32, tag="ln_lnv", bufs=2, name="lnv")
        nc.scalar.activation(lnv, var, AF.Ln, bias=eps_tile, scale=1.0)
        rstd = sbp.tile([1, LT], f32, tag="ln_rstd", bufs=2, name="rstd")
        nc.scalar.activation(rstd, lnv, AF.Exp, scale=-0.5)
        mua = sbp.tile([1, LT], f32, tag="ln_mua", bufs=2, name="mua")
        nc.vector.tensor_tensor(mua, mean, rstd, ALU.mult)
        rstd_h = sbp.tile([1, LT], bf16, tag="ln_rsh", bufs=2, name="rstd_h")
        nc.vector.tensor_copy(out=rstd_h, in_=rstd)
        mua_h = sbp.tile([1, LT], bf16, tag="ln_muh", bufs=2, name="mua_h")
        nc.vector.tensor_copy(out=mua_h, in_=mua)
        a_bc = _bcast_rows(nc, sbp, rstd_h, 128, LT, "a_bc", bf16, 4)
        m_bc = _bcast_rows(nc, sbp, mua_h, 128, LT, "m_bc", bf16, 4)
        return a_bc, m_bc

    def ln_apply(sbp, xm_view, a_bc, m_bc, scale_ap, shift_ap):
        for dc in range(DC):
            t = sbp.tile([128, LT], bf16, tag="ln_t", bufs=2, name="ln_t")
            nc.vector.tensor_tensor(t, xm_view[:, dc, :], a_bc, ALU.mult)
            nc.vector.tensor_tensor(t, t, m_bc, ALU.subtract)
            nc.vector.tensor_scalar(
                out=xm_view[:, dc, :], in0=t,
                scalar1=scale_ap[:, dc:dc + 1], scalar2=shift_ap[:, dc:dc + 1],
                op0=ALU.mult, op1=ALU.add)

    # ====== phase 0+1: time modulation vector + LN1, overlapped ======
    # Order: LN1 stats (PE matmuls + DVE) run first; the tp chunks needed
    # by LN1's apply (shift/scale_msa, Wt cols 0:2048) are computed next;
    # then the applies run on DVE while the PE grinds the remaining tp
    # chunks (only needed from proj onward).
    with ExitStack() as ph:
        sbp = ph.enter_context(tc.tile_pool(name="p01_sb", bufs=2))
        psp = ph.enter_context(tc.tile_pool(name="p01_ps", bufs=1,
                                            space="PSUM"))
        sig = sbp.tile([128, DC], f32, tag="sig", bufs=1, name="sig")
        nc.scalar.activation(sig, bias_sb["temb"], AF.Sigmoid)
        silu_bf = sbp.tile([128, DC], bf16, tag="silu", bufs=1, name="silu_bf")
        nc.vector.tensor_tensor(silu_bf, bias_sb["temb"], sig, ALU.mult)

        def tp_chunks(ps_col0, fb_lo, fb_hi, ps_tag):
            ps_t = psp.tile([128, 4 * (fb_hi - fb_lo)], f32, tag=ps_tag,
                            bufs=1, name=ps_tag)
            for fb in range(fb_lo, fb_hi):
                wt_f = sbp.tile([128, DC, 512], bf16, tag="wt", bufs=2,
                                name="wt_f")
                nc.sync.dma_start(out=wt_f,
                                  in_=wt_r[:, :, fb * 512:(fb + 1) * 512])
                for fl in range(4):
                    f = fb * 4 + fl
                    for dc in range(DC):
                        nc.tensor.matmul(
                            ps_t[:, f - ps_col0:f - ps_col0 + 1],
                            wt_f[:, dc, fl * 128:(fl + 1) * 128],
                            silu_bf[:, dc:dc + 1],
                            start=(dc == 0), stop=(dc == DC - 1))
            return ps_t

        # LN1 stats for all 4 token tiles (tp-independent)
        bcs = []
        for t4 in range(4):
            lts = slice(t4 * LT, (t4 + 1) * LT)
            if t4 < NLT_OWN:  # own half: cast the f32 residual into xmod
                nc.vector.tensor_copy(out=xmod[:, :, lts],
                                      in_=x_own[:, :, lts])
            bcs.append(ln_stats(sbp, psp, xmod[:, :, lts]))

        # tp chunks 0:16 -> shift_msa / scale_msa
        ps_a = tp_chunks(0, 0, 4, "tpa")
        nc.vector.tensor_tensor(tp[:, :16], ps_a, bias_sb["bt"][:, :16],
                                ALU.add)
        nc.vector.tensor_scalar_add(s_msa, tp[:, 8:16], 1.0)

        # LN1 applies (DVE) overlap the remaining tp matmuls (PE)
        for t4 in range(4):
            lts = slice(t4 * LT, (t4 + 1) * LT)
            ln_apply(sbp, xmod[:, :, lts], bcs[t4][0], bcs[t4][1],
                     s_msa, shift_msa)

        ps_b = tp_chunks(16, 4, 12, "tpb")
        nc.vector.tensor_tensor(tp[:, 16:], ps_b, bias_sb["bt"][:, 16:],
                                ALU.add)
        nc.vector.tensor_scalar_add(s_mlp, tp[:, 32:40], 1.0)
        nc.vector.tensor_tensor(gmbp, gate_msa, bias_sb["bproj"], ALU.mult)
        nc.vector.tensor_tensor(gmb2, gate_mlp, bias_sb["b2"], ALU.mult)

    # ================= phase 2: V ================
    # V only; Q/K production is fused into the attention pipeline below.
    nc.vector.memset(v_aug[:, :, :, HD:], 1.0)
    with ExitStack() as ph:
        sbp = ph.enter_context(tc.tile_pool(name="p2_sb", bufs=2))
        psp = ph.enter_context(tc.tile_pool(name="p2_ps", bufs=1, space="PSUM"))
        # V: x-stationary so it lands token-major.
        wv_sb, fr_wv = tc.tile([128, DC, D], bf16, name="wv_sb")
        nc.sync.dma_start(out=wv_sb, in_=wqkv_r[:, :, 2 * D:3 * D])
        for tcn in range(NKC):
            psv = psp.tile([128, 2, LT], f32, tag="v", bufs=2, name="ps_v")
            for dc in range(DC):
                for vs in range(2):
                    nc.tensor.matmul(
                        psv[:, vs, :],
                        xmod[:, dc, tcn * 128:(tcn + 1) * 128],
                        wv_sb[:, dc, vs * LT:(vs + 1) * LT],
                        start=(dc == 0), stop=(dc == DC - 1))
            for vs in range(2):
                nc.vector.tensor_tensor(
                    v_aug[:, tcn, vs * 8:(vs + 1) * 8, :HD],
                    psv[:, vs, :], bv_bc[:, vs * LT:(vs + 1) * LT], ALU.add)
        fr_wv()
    attn_sb, fr_attn = tc.tile([128, DC, LOWN], bf16, name="attn_sb")

    # ================= phase 3: attention (+ fused Q/K production) ========
    # Software-pipelined per key-chunk g: PE runs QK(g) then AV(g-1) while
    # the scalar engine (the bottleneck: 33.5M exp elements) runs exp(g-1).
    # Q/K for head-pair hc+1 are produced inside pair hc's loop, filling the
    # PE slack under the saturated scalar engine; their epilogues run on the
    # otherwise-idle DVE.  PSUM: shared "qk" tag (2 banks x2 bufs) for both
    # score pairs and Q/K producer groups + two av tiles x2 bufs = 8 banks.
    with ExitStack() as ph:
        sbp = ph.enter_context(tc.tile_pool(name="p3_sb", bufs=2))
        psp = ph.enter_context(tc.tile_pool(name="p3_ps", bufs=1, space="PSUM"))

        def qk_producers(hcn):
            """DMA the two weight chunks now; return closures that emit the
            Q (own tokens) and K (all tokens) matmul groups for pair hcn."""
            wq = sbp.tile([128, DC, 128], bf16, tag="wqk", bufs=4, name="wq")
            nc.sync.dma_start(out=wq,
                              in_=wqkv_r[:, :, hcn * 128:(hcn + 1) * 128])
            wk = sbp.tile([128, DC, 128], bf16, tag="wqk", bufs=4, name="wk")
            nc.sync.dma_start(
                out=wk, in_=wqkv_r[:, :, (8 + hcn) * 128:(9 + hcn) * 128])

            def q_group():
                ps = psp.tile([128, 2, LT], f32, tag="qk", bufs=2, name="ps_q")
                for dc in range(DC):
                    for j in range(2):
                        nc.tensor.matmul(
                            ps[:, j, :], wq[:, dc, :],
                            xmod[:, dc, j * LT:(j + 1) * LT],
                            start=(dc == 0), stop=(dc == DC - 1))
                for j in range(2):
                    nc.vector.tensor_scalar(
                        out=q_sb[:, hcn, j * LT:(j + 1) * LT], in0=ps[:, j, :],
                        scalar1=0.125, scalar2=bias_sb["bq"][:, hcn:hcn + 1],
                        op0=ALU.mult, op1=ALU.add)

            def k_group(g2):
                ps = psp.tile([128, 2, LT], f32, tag="qk", bufs=2, name="ps_k")
                for dc in range(DC):
                    for j in range(2):
                        lt = 2 * g2 + j
                        nc.tensor.matmul(
                            ps[:, j, :], wk[:, dc, :],
                            xmod[:, dc, lt * LT:(lt + 1) * LT],
                            start=(dc == 0), stop=(dc == DC - 1))
                for j in range(2):
                    lt = 2 * g2 + j
                    nc.vector.tensor_scalar(
                        out=k_sb[:, hcn, lt * LT:(lt + 1) * LT],
                        in0=ps[:, j, :],
                        scalar1=bias_sb["bk"][:, hcn:hcn + 1], scalar2=None,
                        op0=ALU.add)

            return [q_group, lambda: k_group(0), lambda: k_group(1)]

        def emit_qk_exp(hc, lt, g):
            lts = slice(lt * LT, (lt + 1) * LT)
            ms = slice(g * 128, (g + 1) * 128)
            ps_pair = psp.tile([128, 2, LT], f32, tag="qk", bufs=2,
                               name="ps_pair")
            nc.tensor.matmul(ps_pair[:, 0, :], k_sb[0:64, hc, ms],
                             q_sb[0:64, hc, lts],
                             start=True, stop=True, tile_position=(0, 0))
            nc.tensor.matmul(ps_pair[:, 1, :], k_sb[64:128, hc, ms],
                             q_sb[64:128, hc, lts],
                             start=True, stop=True, tile_position=(64, 0))
            ept = sbp.tile([128, 2, LT], bf16, tag="ept", bufs=3, name="ept")
            nc.scalar.activation(ept, ps_pair, AF.Exp)
            return ept

        # prologue: produce pair 0 before its scores are needed
        for op in qk_producers(0):
            op()
        for hc in range(H // 2):
            pend = qk_producers(hc + 1) if hc + 1 < H // 2 else []
            for lt in range(NLT_OWN):
                lts = slice(lt * LT, (lt + 1) * LT)
                ps_av = [psp.tile([HD + 1, LT], f32, tag=f"av{i}", bufs=2,
                                  name=f"ps_av{i}") for i in range(2)]
                prev = emit_qk_exp(hc, lt, 0)
                for g in range(1, NKC):
                    cur = emit_qk_exp(hc, lt, g)
                    for i in range(2):
                        nc.tensor.matmul(ps_av[i], v_aug[:, g - 1, 2 * hc + i, :],
                                         prev[:, i, :],
                                         start=(g == 1), stop=False)
                    prev = cur
                    if pend and (lt * NKC + g) % 9 == 8:
                        pend.pop(0)()
                for i in range(2):
                    nc.tensor.matmul(ps_av[i], v_aug[:, NKC - 1, 2 * hc + i, :],
                                     prev[:, i, :], start=False, stop=True)
                for i in range(2):
                    rcp = sbp.tile([1, LT], f32, tag="rcp", bufs=2, name="rcp")
                    nc.vector.reciprocal(out=rcp, in_=ps_av[i][HD:HD + 1, :])
                    rcp_bc = _bcast_rows(nc, sbp, rcp, 64, LT, "rcp_bc")
                    if i == 0:
                        nc.vector.tensor_tensor(
                            attn_sb[0:64, hc, lts],
                            ps_av[0][:HD, :], rcp_bc, ALU.mult)
                    else:
                        # DVE lanes can't shift partitions; bounce head 1
                        # through a small SBUF->SBUF DMA instead of DRAM.
                        at = sbp.tile([64, LT], bf16, tag="at", bufs=2,
                                      name="at")
                        nc.vector.tensor_tensor(at, ps_av[1][:HD, :], rcp_bc,
                                                ALU.mult)
                        nc.sync.dma_start(out=attn_sb[64:128, hc, lts],
                                          in_=at)

    # ================= phase 4: proj + residual ================
    with ExitStack() as ph:
        sbp = ph.enter_context(tc.tile_pool(name="p4_sb", bufs=2))
        psp = ph.enter_context(tc.tile_pool(name="p4_ps", bufs=1, space="PSUM"))
        wp_all = sbp.tile([128, DC, D], bf16, tag="wpj", bufs=1, name="wp_all")
        nc.sync.dma_start(out=wp_all, in_=wproj_r)
        for ft in range(DC):
            ps = [psp.tile([128, LT], f32, tag=f"pj{i}", bufs=2,
                           name=f"ps_pj{i}") for i in range(NLT_OWN)]
            for dc in range(DC):
                for lt in range(NLT_OWN):
                    nc.tensor.matmul(
                        ps[lt], wp_all[:, dc, ft * 128:(ft + 1) * 128],
                        attn_sb[:, dc, lt * LT:(lt + 1) * LT],
                        start=(dc == 0), stop=(dc == DC - 1))
            for lt in range(NLT_OWN):
                gh = sbp.tile([128, LT], f32, tag="gh", bufs=3, name="gh")
                nc.scalar.activation(gh, ps[lt], AF.Identity,
                                     bias=gmbp[:, ft:ft + 1],
                                     scale=gate_msa[:, ft:ft + 1])
                xo = x_own[:, ft, lt * LT:(lt + 1) * LT]
                nc.vector.tensor_tensor(xo, xo, gh, ALU.add)
    fr_attn()
    fr_xmod()
    fr_q()
    fr_v()
    fr_k()

    # ================= phase 5/6: LN2 + MLP ================
    gelu_sb, fr_gelu = tc.tile([128, MC, LOWN], bf16, name="gelu_sb")
    h2mod, fr_h2 = tc.tile([128, DC, LOWN], bf16, name="h2mod")
    with ExitStack() as ph:
        sbp = ph.enter_context(tc.tile_pool(name="p5_sb", bufs=2))
        psp = ph.enter_context(tc.tile_pool(name="p5_ps", bufs=1, space="PSUM"))
        for lt in range(NLT_OWN):
            lts = slice(lt * LT, (lt + 1) * LT)
            nc.vector.tensor_copy(out=h2mod[:, :, lts], in_=x_own[:, :, lts])
            a_bc, m_bc = ln_stats(sbp, psp, h2mod[:, :, lts])
            ln_apply(sbp, h2mod[:, :, lts], a_bc, m_bc, s_mlp, shift_mlp)

    with ExitStack() as ph:
        sbp = ph.enter_context(tc.tile_pool(name="p6_sb", bufs=2))
        psp = ph.enter_context(tc.tile_pool(name="p6_ps", bufs=1, space="PSUM"))
        for ft in range(MC):
            wti = sbp.tile([128, DC, 128], bf16, tag="w1t", bufs=4, name="w1t")
            nc.sync.dma_start(out=wti,
                              in_=w1_r[:, :, ft * 128:(ft + 1) * 128])
            ps = [psp.tile([128, LT], f32, tag=f"f1{i}", bufs=2,
                           name=f"ps_f1{i}") for i in range(NLT_OWN)]
            for dc in range(DC):
                for lt in range(NLT_OWN):
                    nc.tensor.matmul(
                        ps[lt], wti[:, dc, :],
                        h2mod[:, dc, lt * LT:(lt + 1) * LT],
                        start=(dc == 0), stop=(dc == DC - 1))
            for lt in range(NLT_OWN):
                nc.scalar.activation(
                    gelu_sb[:, ft, lt * LT:(lt + 1) * LT], ps[lt], AF.Gelu,
                    bias=bias_sb["b1"][:, ft:ft + 1])
    fr_h2()

    with ExitStack() as ph:
        sbp = ph.enter_context(tc.tile_pool(name="p7_sb", bufs=2))
        psp = ph.enter_context(tc.tile_pool(name="p7_ps", bufs=1, space="PSUM"))
        for ft in range(DC):
            wti = sbp.tile([128, MC, 128], bf16, tag="w2t", bufs=2, name="w2t")
            nc.sync.dma_start(out=wti,
                              in_=w2_r[:, :, ft * 128:(ft + 1) * 128])
            ps = [psp.tile([128, LT], f32, tag=f"f2{i}", bufs=2,
                           name=f"ps_f2{i}") for i in range(NLT_OWN)]
            for mc in range(MC):
                for lt in range(NLT_OWN):
                    nc.tensor.matmul(
                        ps[lt], wti[:, mc, :],
                        gelu_sb[:, mc, lt * LT:(lt + 1) * LT],
                        start=(mc == 0), stop=(mc == MC - 1))
            for lt in range(NLT_OWN):
                gh = sbp.tile([128, LT], f32, tag="gh2", bufs=3, name="gh2")
                nc.scalar.activation(gh, ps[lt], AF.Identity,
                                     bias=gmb2[:, ft:ft + 1],
                                     scale=gate_mlp[:, ft:ft + 1])
                xo = x_own[:, ft, lt * LT:(lt + 1) * LT]
                nc.vector.tensor_tensor(xo, xo, gh, ALU.add)
    fr_gelu()

    # ================= output ================
    outr = out.rearrange("(c p) t -> p c t", p=128)
    nc.sync.dma_start(out=outr, in_=x_own)

    # release persistents
    fr_x_own()
    fr_g2(); fr_g1(); fr_s2(); fr_s1(); fr_tp()
    fr_bv()
    for fr in reversed(bias_frees):
        fr()
    fr_eps(); fr_ones_bf()


_PROGRAM_CACHE = {}


def _get_program():
    if "nc" not in _PROGRAM_CACHE:
        _PROGRAM_CACHE["nc"] = build_program()
    return _PROGRAM_CACHE["nc"]


def _fm(v):
    """[D] vector -> feature-major [128, D//128] (partition p, chunk c)."""
    return np.ascontiguousarray(np.asarray(v, np.float32).reshape(-1, 128).T)


def make_in_maps(x, time_emb, Wqkv, bqkv, Wproj, bproj, W1, b1, W2, b2, Wt, bt,
                 g1, be1, g2, be2):
    # g1/be1/g2/be2 are identity layernorm params in this module; verify and
    # fold them away.
    assert np.allclose(g1, 1.0) and np.allclose(g2, 1.0)
    assert np.allclose(be1, 0.0) and np.allclose(be2, 0.0)

    x = np.asarray(x, np.float32)
    shared = {
        "wqkv": np.asarray(Wqkv, np.float32).astype(BF),
        "bq": _fm(np.asarray(bqkv[:D]) * 0.125),
        "bk": _fm(bqkv[D:2 * D]),
        "bv": np.ascontiguousarray(np.asarray(bqkv[2 * D:], np.float32)[None, :]),
        "wproj": np.asarray(Wproj, np.float32).astype(BF),
        "bproj": _fm(bproj),
        "w1": np.asarray(W1, np.float32).astype(BF),
        "b1": _fm(b1),
        "w2": np.asarray(W2, np.float32).astype(BF),
        "b2": _fm(b2),
        "wt": np.asarray(Wt, np.float32).astype(BF),
        "bt": _fm(bt),
    }
    in_maps = []
    for c in range(NCORES):
        b, half = c // 2, c % 2
        xb = x[b].T  # [D, L] feature-major
        own = slice(half * LOWN, (half + 1) * LOWN)
        oth = slice((1 - half) * LOWN, (2 - half) * LOWN)
        m = dict(shared)
        m["xfm"] = np.ascontiguousarray(xb[:, own])
        m["xoth"] = np.ascontiguousarray(xb[:, oth]).astype(BF)
        m["temb"] = _fm(time_emb[b])
        in_maps.append(m)
    return in_maps


def assemble_output(results):
    outp = np.empty((B, L, D), np.float32)
    for c in range(NCORES):
        b, half = c // 2, c % 2
        outp[b, half * LOWN:(half + 1) * LOWN, :] = results[c]["out_fm"].T
    return outp


def kernel(x, time_emb, Wqkv, bqkv, Wproj, bproj, W1, b1, W2, b2, Wt, bt,
           g1, be1, g2, be2, trace=False, trace_kwargs=None):
    in_maps = make_in_maps(x, time_emb, Wqkv, bqkv, Wproj, bproj, W1, b1,
                           W2, b2, Wt, bt, g1, be1, g2, be2)
    nc = _get_program()
    res = run_bass_kernel_spmd(nc, in_maps, core_ids=list(range(NCORES)),
                               trace=trace, trace_kwargs=trace_kwargs or {})
    kernel.last_results = res
    return assemble_output(res.results)


# revision 26
# speedup vs baseline: 1.0669x; 1.0669x over previous
"""DiT block (adaLN) Trainium2 kernel, 8-core SPMD, no collectives.

Sharding: core c handles batch b = c//2 and query-token half c%2 (1024 q
tokens).  Each core computes K/V for all 2048 tokens of its batch (the
only duplicated work), so cores never communicate.  The host permutes
each core's token columns so its own 1024 tokens come first (softmax is
invariant to key order), and transposes x to feature-major [D, L] so the
device never transposes anything.

On-device layout is feature-major everywhere: activations live as
[128 partitions, d-chunk, tokens].  LayerNorm stats (per-token = free
dim) are computed with ones-vector matmuls on the tensor engine and
broadcast back across partitions with stride-0 DMA.  All GEMM operands
are bf16 (fp32 PSUM accumulation); the residual stream, softmax and LN
statistics stay fp32.

Perf structure (vs the first working version):
- All weights are staged into SBUF with large multi-dim DMAs (one per
  output-feature chunk) instead of one DMA per 128x128 tile; the small
  DMAs serialized on the sync sequencer (~600ns each, ~1150 of them).
- Attention is software-pipelined at key-chunk granularity: the PE does
  QK(g+1) and AV(g-1) while the scalar engine does exp(g), so the PE
  never idles long enough for the HAM clock gate to re-throttle.
- Attention output is written straight into SBUF (partition-offset
  writes per head) instead of bouncing through DRAM.
"""

import os
import sys
from contextlib import ExitStack

os.environ.setdefault("MYCRO_LOCAL_CACHE", "1")
for _p in ("/opt/trn_rl_repo", "/root/.axon_site/_ro/trn_rl_repo"):
    if os.path.isdir(_p) and _p not in sys.path:
        sys.path.insert(0, _p)

import ml_dtypes
import numpy as np

import concourse.bass as bass
import concourse.tile as tile
from concourse import bacc, mybir
from concourse.bass_utils import run_bass_kernel_spmd

B, L, D, H, HD, MLPD = 4, 2048, 1024, 16, 64, 4096
NCORES = 8
LOWN = L // 2          # own query tokens per core
DC = D // 128          # 8 chunks of the model dim
MC = MLPD // 128       # 32 chunks of the mlp dim
LT = 512               # token tile for matmul free dim
NLT_OWN = LOWN // LT   # 2 token tiles (queries)
NKC = L // 128         # 16 key chunks

f32 = mybir.dt.float32
bf16 = mybir.dt.bfloat16
AF = mybir.ActivationFunctionType
ALU = mybir.AluOpType
BF = ml_dtypes.bfloat16


def _bcast_rows(nc, pool, row_ap, nrows, ncols, tag, dtype=None, bufs=2):
    """SBUF [nrows, ncols] tile = row_ap ([1, ncols] SBUF) broadcast
    across partitions, on the otherwise-idle GpSimd engine."""
    dtype = dtype or mybir.dt.float32
    out = pool.tile([nrows, ncols], dtype, tag=tag, bufs=bufs, name=tag)
    nc.gpsimd.partition_broadcast(out, row_ap)
    return out


def build_program():
    # Bacc (not plain Bass): its compile() pass legalizes multi-semaphore
    # waits (event semaphores, nop fusion) that walrus can't encode raw.
    nc = bacc.Bacc()

    def _in(name, shape, dtype):
        return nc.declare_dram_parameter(name, shape, dtype, False)[:]

    xfm = _in("xfm", [D, LOWN], f32)
    xoth = _in("xoth", [D, LOWN], bf16)
    temb = _in("temb", [128, DC], f32)
    wqkv = _in("wqkv", [D, 3 * D], bf16)
    bq = _in("bq", [128, DC], f32)     # pre-scaled by 1/8
    bk = _in("bk", [128, DC], f32)
    bv = _in("bv", [1, D], f32)
    wproj = _in("wproj", [D, D], bf16)
    bproj = _in("bproj", [128, DC], f32)
    w1 = _in("w1", [D, MLPD], bf16)
    b1 = _in("b1", [128, MC], f32)
    w2 = _in("w2", [MLPD, D], bf16)
    b2 = _in("b2", [128, DC], f32)
    wt = _in("wt", [D, 6 * D], bf16)
    bt = _in("bt", [128, 48], f32)
    out = nc.declare_dram_parameter("out_fm", [D, LOWN], f32, True)[:]

    with tile.TileContext(nc) as tc:
        _emit_kernel(tc, xfm, xoth, temb, wqkv, bq, bk, bv, wproj, bproj,
                     w1, b1, w2, b2, wt, bt, out)
    nc.finalize()  # runs Bacc.compile(): reg alloc + sync legalization
    return nc


def _emit_kernel(tc, xfm, xoth, temb, wqkv, bq, bk, bv, wproj, bproj, w1, b1,
                 w2, b2, wt, bt, out):
    nc = tc.nc

    # feature-major views of the weight matrices: [128, in-chunk, out-col]
    wt_r = wt.rearrange("(c p) n -> p c n", p=128)
    wqkv_r = wqkv.rearrange("(c p) n -> p c n", p=128)
    wproj_r = wproj.rearrange("(c p) n -> p c n", p=128)
    w1_r = w1.rearrange("(c p) n -> p c n", p=128)
    w2_r = w2.rearrange("(c p) n -> p c n", p=128)

    # ---- persistent constants / host-prepped vectors (freed last) ----
    ones_bf, fr_ones_bf = tc.tile([128, 1], bf16, name="ones_bf")
    nc.vector.memset(ones_bf, 1.0)
    eps_tile, fr_eps = tc.tile([1, 1], f32, name="eps_tile")
    nc.vector.memset(eps_tile, 1e-5)

    bias_sb = {}
    bias_frees = []
    for name, ap, w in (("bq", bq, DC), ("bk", bk, DC), ("bproj", bproj, DC),
                        ("b1", b1, MC), ("b2", b2, DC), ("bt", bt, 48),
                        ("temb", temb, DC)):
        t, fr = tc.tile([128, w], f32, name=f"sb_{name}")
        nc.sync.dma_start(out=t, in_=ap)
        bias_sb[name] = t
        bias_frees.append(fr)
    bv_bc, fr_bv = tc.tile([128, D], f32, name="bv_bc")
    nc.sync.dma_start(
        out=bv_bc,
        in_=bass.AP(tensor=bv.tensor, offset=bv.offset,
                    ap=[[0, 128]] + [list(x) for x in bv.ap[1:]]))

    # modulation vectors (computed in phase 0, consumed later)
    tp, fr_tp = tc.tile([128, 48], f32, name="tp")
    s_msa, fr_s1 = tc.tile([128, DC], f32, name="s_msa")
    s_mlp, fr_s2 = tc.tile([128, DC], f32, name="s_mlp")
    gmbp, fr_g1 = tc.tile([128, DC], f32, name="gmbp")
    gmb2, fr_g2 = tc.tile([128, DC], f32, name="gmb2")
    shift_msa = tp[:, 0:8]
    gate_msa = tp[:, 16:24]
    shift_mlp = tp[:, 24:32]
    gate_mlp = tp[:, 40:48]

    # ---- big persistent activations ----
    x_own, fr_x_own = tc.tile([128, DC, LOWN], f32, name="x_own")
    k_sb, fr_k = tc.tile([128, DC, L], bf16, name="k_sb")
    # v_aug: [token-part, token-chunk, head, 65]; col 64 holds ones so the
    # AV matmul also produces the softmax denominator.
    v_aug, fr_v = tc.tile([128, NKC, H, HD + 1], bf16, name="v_aug")
    q_sb, fr_q = tc.tile([128, DC, LOWN], bf16, name="q_sb")
    xmod, fr_xmod = tc.tile([128, DC, L], bf16, name="xmod")

    xr = xfm.rearrange("(c p) t -> p c t", p=128)
    nc.sync.dma_start(out=x_own, in_=xr)
    # other token half arrives pre-cast to bf16 and lands directly in xmod;
    # LN1 then runs in place on it.
    xor_ = xoth.rearrange("(c p) t -> p c t", p=128)
    nc.sync.dma_start(out=xmod[:, :, LOWN:], in_=xor_)

    # ---- LayerNorm helpers: bf16, in place on a [128, DC, LT] bf16 tile ----
    def ln_stats(sbp, psp, xm_view):
        """Stats over the (pre-modulation) bf16 tile; returns broadcast
        tiles (a_bc=rstd, m_bc=mu*rstd) in bf16."""
        ps_s = psp.tile([1, LT], f32, tag="st_s", bufs=2, name="ps_s")
        ps_q = psp.tile([1, LT], f32, tag="st_q", bufs=2, name="ps_q")
        for dc in range(DC):
            xs = xm_view[:, dc, :]
            nc.tensor.matmul(ps_s, ones_bf, xs,
                             start=(dc == 0), stop=(dc == DC - 1))
            sq = sbp.tile([128, LT], bf16, tag="sq", bufs=2, name="sq")
            nc.vector.tensor_tensor(sq, xs, xs, ALU.mult)
            nc.tensor.matmul(ps_q, ones_bf, sq,
                             start=(dc == 0), stop=(dc == DC - 1))
        mean = sbp.tile([1, LT], f32, tag="ln_mean", bufs=2, name="mean")
        var = sbp.tile([1, LT], f32, tag="ln_var", bufs=2, name="var")
        msq = sbp.tile([1, LT], f32, tag="ln_msq", bufs=2, name="msq")
        nc.vector.tensor_scalar_mul(mean, ps_s, 1.0 / D)
        nc.vector.tensor_scalar_mul(var, ps_q, 1.0 / D)
        nc.vector.tensor_tensor(msq, mean, mean, ALU.mult)
        nc.vector.tensor_tensor(var, var, msq, ALU.subtract)
        # rstd = (var+eps)^-0.5 as Exp(-0.5*Ln(var+eps)): both functions sit
        # in one scalar table set, and it avoids the slow DVE reciprocal.
        lnv = sbp.tile([1, LT], f32, tag="ln_lnv", bufs=2, name="lnv")
        nc.scalar.activation(lnv, var, AF.Ln, bias=eps_tile, scale=1.0)
        rstd = sbp.tile([1, LT], f32, tag="ln_rstd", bufs=2, name="rstd")
        nc.scalar.activation(rstd, lnv, AF.Exp, scale=-0.5)
        mua = sbp.tile([1, LT], f32, tag="ln_mua", bufs=2, name="mua")
        nc.vector.tensor_tensor(mua, mean, rstd, ALU.mult)
        rstd_h = sbp.tile([1, LT], bf16, tag="ln_rsh", bufs=2, name="rstd_h")
        nc.vector.tensor_copy(out=rstd_h, in_=rstd)
        mua_h = sbp.tile([1, LT], bf16, tag="ln_muh", bufs=2, name="mua_h")
        nc.vector.tensor_copy(out=mua_h, in_=mua)
        a_bc = _bcast_rows(nc, sbp, rstd_h, 128, LT, "a_bc", bf16, 4)
        m_bc = _bcast_rows(nc, sbp, mua_h, 128, LT, "m_bc", bf16, 4)
        return a_bc, m_bc

    def ln_apply(sbp, xm_view, a_bc, m_bc, scale_ap, shift_ap):
        for dc in range(DC):
            t = sbp.tile([128, LT], bf16, tag="ln_t", bufs=2, name="ln_t")
            nc.vector.tensor_tensor(t, xm_view[:, dc, :], a_bc, ALU.mult)
            nc.vector.tensor_tensor(t, t, m_bc, ALU.subtract)
            nc.vector.tensor_scalar(
                out=xm_view[:, dc, :], in0=t,
                scalar1=scale_ap[:, dc:dc + 1], scalar2=shift_ap[:, dc:dc + 1],
                op0=ALU.mult, op1=ALU.add)

    # ====== phase 0+1: time modulation vector + LN1, overlapped ======
    # Order: LN1 stats (PE matmuls + DVE) run first; the tp chunks needed
    # by LN1's apply (shift/scale_msa, Wt cols 0:2048) are computed next;
    # then the applies run on DVE while the PE grinds the remaining tp
    # chunks (only needed from proj onward).
    with ExitStack() as ph:
        sbp = ph.enter_context(tc.tile_pool(name="p01_sb", bufs=2))
        psp = ph.enter_context(tc.tile_pool(name="p01_ps", bufs=1,
                                            space="PSUM"))
        sig = sbp.tile([128, DC], f32, tag="sig", bufs=1, name="sig")
        nc.scalar.activation(sig, bias_sb["temb"], AF.Sigmoid)
        silu_bf = sbp.tile([128, DC], bf16, tag="silu", bufs=1, name="silu_bf")
        nc.vector.tensor_tensor(silu_bf, bias_sb["temb"], sig, ALU.mult)

        def tp_chunks(ps_col0, fb_lo, fb_hi, ps_tag):
            ps_t = psp.tile([128, 4 * (fb_hi - fb_lo)], f32, tag=ps_tag,
                            bufs=1, name=ps_tag)
            for fb in range(fb_lo, fb_hi):
                wt_f = sbp.tile([128, DC, 512], bf16, tag="wt", bufs=2,
                                name="wt_f")
                nc.sync.dma_start(out=wt_f,
                                  in_=wt_r[:, :, fb * 512:(fb + 1) * 512])
                for fl in range(4):
                    f = fb * 4 + fl
                    for dc in range(DC):
                        nc.tensor.matmul(
                            ps_t[:, f - ps_col0:f - ps_col0 + 1],
                            wt_f[:, dc, fl * 128:(fl + 1) * 128],
                            silu_bf[:, dc:dc + 1],
                            start=(dc == 0), stop=(dc == DC - 1))
            return ps_t

        # LN1 stats for all 4 token tiles (tp-independent)
        bcs = []
        for t4 in range(4):
            lts = slice(t4 * LT, (t4 + 1) * LT)
            if t4 < NLT_OWN:  # own half: cast the f32 residual into xmod
                nc.vector.tensor_copy(out=xmod[:, :, lts],
                                      in_=x_own[:, :, lts])
            bcs.append(ln_stats(sbp, psp, xmod[:, :, lts]))

        # tp chunks 0:16 -> shift_msa / scale_msa
        ps_a = tp_chunks(0, 0, 4, "tpa")
        nc.vector.tensor_tensor(tp[:, :16], ps_a, bias_sb["bt"][:, :16],
                                ALU.add)
        nc.vector.tensor_scalar_add(s_msa, tp[:, 8:16], 1.0)

        # LN1 applies (DVE) overlap the remaining tp matmuls (PE)
        for t4 in range(4):
            lts = slice(t4 * LT, (t4 + 1) * LT)
            ln_apply(sbp, xmod[:, :, lts], bcs[t4][0], bcs[t4][1],
                     s_msa, shift_msa)

        ps_b = tp_chunks(16, 4, 12, "tpb")
        nc.vector.tensor_tensor(tp[:, 16:], ps_b, bias_sb["bt"][:, 16:],
                                ALU.add)
        nc.vector.tensor_scalar_add(s_mlp, tp[:, 32:40], 1.0)
        nc.vector.tensor_tensor(gmbp, gate_msa, bias_sb["bproj"], ALU.mult)
        nc.vector.tensor_tensor(gmb2, gate_mlp, bias_sb["b2"], ALU.mult)

    # ================= phase 2: QKV ================
    nc.vector.memset(v_aug[:, :, :, HD:], 1.0)
    with ExitStack() as ph:
        sbp = ph.enter_context(tc.tile_pool(name="p2_sb", bufs=2))
        psp = ph.enter_context(tc.tile_pool(name="p2_ps", bufs=1, space="PSUM"))
        # Q and K: weight-stationary.  ft 0..7 -> q (own tokens only),
        # ft 8..15 -> k (all tokens).
        for ft in range(16):
            is_q = ft < 8
            nlt = NLT_OWN if is_q else L // LT
            wti = sbp.tile([128, DC, 128], bf16, tag="wqk", bufs=3, name="wti")
            nc.sync.dma_start(out=wti,
                              in_=wqkv_r[:, :, ft * 128:(ft + 1) * 128])
            for lt0 in range(0, nlt, 2):
                nl = min(2, nlt - lt0)
                ps = [psp.tile([128, LT], f32, tag=f"qk{i}", bufs=2,
                               name=f"ps_qk{i}") for i in range(nl)]
                for dc in range(DC):
                    for i in range(nl):
                        lt = lt0 + i
                        nc.tensor.matmul(
                            ps[i], wti[:, dc, :],
                            xmod[:, dc, lt * LT:(lt + 1) * LT],
                            start=(dc == 0), stop=(dc == DC - 1))
                for i in range(nl):
                    lt = lt0 + i
                    if is_q:
                        nc.scalar.activation(
                            q_sb[:, ft, lt * LT:(lt + 1) * LT], ps[i],
                            AF.Identity, bias=bias_sb["bq"][:, ft:ft + 1],
                            scale=0.125)
                    else:
                        nc.scalar.activation(
                            k_sb[:, ft - 8, lt * LT:(lt + 1) * LT], ps[i],
                            AF.Identity, bias=bias_sb["bk"][:, ft - 8:ft - 7])

        # V: x-stationary so it lands token-major.
        wv_sb, fr_wv = tc.tile([128, DC, D], bf16, name="wv_sb")
        nc.sync.dma_start(out=wv_sb, in_=wqkv_r[:, :, 2 * D:3 * D])
        for tcn in range(NKC):
            psv = psp.tile([128, 2, LT], f32, tag="v", bufs=2, name="ps_v")
            for dc in range(DC):
                for vs in range(2):
                    nc.tensor.matmul(
                        psv[:, vs, :],
                        xmod[:, dc, tcn * 128:(tcn + 1) * 128],
                        wv_sb[:, dc, vs * LT:(vs + 1) * LT],
                        start=(dc == 0), stop=(dc == DC - 1))
            for vs in range(2):
                nc.vector.tensor_tensor(
                    v_aug[:, tcn, vs * 8:(vs + 1) * 8, :HD],
                    psv[:, vs, :], bv_bc[:, vs * LT:(vs + 1) * LT], ALU.add)
        fr_wv()
    fr_xmod()
    attn_sb, fr_attn = tc.tile([128, DC, LOWN], bf16, name="attn_sb")

    # ================= phase 3: attention ================
    # Software-pipelined per key-chunk g: PE runs QK(g) then AV(g-1) while
    # the scalar engine (the bottleneck: 33.5M exp elements) runs exp(g-1).
    # PSUM budget: qk pair tile 2 banks x2 bufs + two av tiles x2 bufs = 8.
    with ExitStack() as ph:
        sbp = ph.enter_context(tc.tile_pool(name="p3_sb", bufs=2))
        psp = ph.enter_context(tc.tile_pool(name="p3_ps", bufs=1, space="PSUM"))

        def emit_qk_exp(hc, lt, g):
            lts = slice(lt * LT, (lt + 1) * LT)
            ms = slice(g * 128, (g + 1) * 128)
            ps_pair = psp.tile([128, 2, LT], f32, tag="qk", bufs=2,
                               name="ps_pair")
            nc.tensor.matmul(ps_pair[:, 0, :], k_sb[0:64, hc, ms],
                             q_sb[0:64, hc, lts],
                             start=True, stop=True, tile_position=(0, 0))
            nc.tensor.matmul(ps_pair[:, 1, :], k_sb[64:128, hc, ms],
                             q_sb[64:128, hc, lts],
                             start=True, stop=True, tile_position=(64, 0))
            ept = sbp.tile([128, 2, LT], bf16, tag="ept", bufs=3, name="ept")
            nc.scalar.activation(ept, ps_pair, AF.Exp)
            return ept

        for hc in range(H // 2):
            for lt in range(NLT_OWN):
                lts = slice(lt * LT, (lt + 1) * LT)
                ps_av = [psp.tile([HD + 1, LT], f32, tag=f"av{i}", bufs=2,
                                  name=f"ps_av{i}") for i in range(2)]
                prev = emit_qk_exp(hc, lt, 0)
                for g in range(1, NKC):
                    cur = emit_qk_exp(hc, lt, g)
                    for i in range(2):
                        nc.tensor.matmul(ps_av[i], v_aug[:, g - 1, 2 * hc + i, :],
                                         prev[:, i, :],
                                         start=(g == 1), stop=False)
                    prev = cur
                for i in range(2):
                    nc.tensor.matmul(ps_av[i], v_aug[:, NKC - 1, 2 * hc + i, :],
                                     prev[:, i, :], start=False, stop=True)
                for i in range(2):
                    rcp = sbp.tile([1, LT], f32, tag="rcp", bufs=2, name="rcp")
                    nc.vector.reciprocal(out=rcp, in_=ps_av[i][HD:HD + 1, :])
                    rcp_bc = _bcast_rows(nc, sbp, rcp, 64, LT, "rcp_bc")
                    if i == 0:
                        nc.vector.tensor_tensor(
                            attn_sb[0:64, hc, lts],
                            ps_av[0][:HD, :], rcp_bc, ALU.mult)
                    else:
                        # DVE lanes can't shift partitions; bounce head 1
                        # through a small SBUF->SBUF DMA instead of DRAM.
                        at = sbp.tile([64, LT], bf16, tag="at", bufs=2,
                                      name="at")
                        nc.vector.tensor_tensor(at, ps_av[1][:HD, :], rcp_bc,
                                                ALU.mult)
                        nc.sync.dma_start(out=attn_sb[64:128, hc, lts],
                                          in_=at)

    # ================= phase 4: proj + residual ================
    with ExitStack() as ph:
        sbp = ph.enter_context(tc.tile_pool(name="p4_sb", bufs=2))
        psp = ph.enter_context(tc.tile_pool(name="p4_ps", bufs=1, space="PSUM"))
        wp_all = sbp.tile([128, DC, D], bf16, tag="wpj", bufs=1, name="wp_all")
        nc.sync.dma_start(out=wp_all, in_=wproj_r)
        for ft in range(DC):
            ps = [psp.tile([128, LT], f32, tag=f"pj{i}", bufs=2,
                           name=f"ps_pj{i}") for i in range(NLT_OWN)]
            for dc in range(DC):
                for lt in range(NLT_OWN):
                    nc.tensor.matmul(
                        ps[lt], wp_all[:, dc, ft * 128:(ft + 1) * 128],
                        attn_sb[:, dc, lt * LT:(lt + 1) * LT],
                        start=(dc == 0), stop=(dc == DC - 1))
            for lt in range(NLT_OWN):
                gh = sbp.tile([128, LT], f32, tag="gh", bufs=3, name="gh")
                nc.scalar.activation(gh, ps[lt], AF.Identity,
                                     bias=gmbp[:, ft:ft + 1],
                                     scale=gate_msa[:, ft:ft + 1])
                xo = x_own[:, ft, lt * LT:(lt + 1) * LT]
                nc.vector.tensor_tensor(xo, xo, gh, ALU.add)
    fr_attn()
    fr_q()
    fr_v()
    fr_k()

    outr = out.rearrange("(c p) t -> p c t", p=128)

    # ================= phase 5/6: LN2 + MLP ================
    gelu_sb, fr_gelu = tc.tile([128, MC, LOWN], bf16, name="gelu_sb")
    h2mod, fr_h2 = tc.tile([128, DC, LOWN], bf16, name="h2mod")
    with ExitStack() as ph:
        sbp = ph.enter_context(tc.tile_pool(name="p5_sb", bufs=2))
        psp = ph.enter_context(tc.tile_pool(name="p5_ps", bufs=1, space="PSUM"))
        for lt in range(NLT_OWN):
            lts = slice(lt * LT, (lt + 1) * LT)
            nc.vector.tensor_copy(out=h2mod[:, :, lts], in_=x_own[:, :, lts])
            a_bc, m_bc = ln_stats(sbp, psp, h2mod[:, :, lts])
            ln_apply(sbp, h2mod[:, :, lts], a_bc, m_bc, s_mlp, shift_mlp)

    with ExitStack() as ph:
        sbp = ph.enter_context(tc.tile_pool(name="p6_sb", bufs=2))
        psp = ph.enter_context(tc.tile_pool(name="p6_ps", bufs=1, space="PSUM"))
        for ft in range(MC):
            wti = sbp.tile([128, DC, 128], bf16, tag="w1t", bufs=4, name="w1t")
            nc.sync.dma_start(out=wti,
                              in_=w1_r[:, :, ft * 128:(ft + 1) * 128])
            ps = [psp.tile([128, LT], f32, tag=f"f1{i}", bufs=2,
                           name=f"ps_f1{i}") for i in range(NLT_OWN)]
            for dc in range(DC):
                for lt in range(NLT_OWN):
                    nc.tensor.matmul(
                        ps[lt], wti[:, dc, :],
                        h2mod[:, dc, lt * LT:(lt + 1) * LT],
                        start=(dc == 0), stop=(dc == DC - 1))
            for lt in range(NLT_OWN):
                nc.scalar.activation(
                    gelu_sb[:, ft, lt * LT:(lt + 1) * LT], ps[lt], AF.Gelu,
                    bias=bias_sb["b1"][:, ft:ft + 1])
    fr_h2()

    with ExitStack() as ph:
        sbp = ph.enter_context(tc.tile_pool(name="p7_sb", bufs=2))
        psp = ph.enter_context(tc.tile_pool(name="p7_ps", bufs=1, space="PSUM"))
        for ft in range(DC):
            wti = sbp.tile([128, MC, 128], bf16, tag="w2t", bufs=2, name="w2t")
            nc.sync.dma_start(out=wti,
                              in_=w2_r[:, :, ft * 128:(ft + 1) * 128])
            ps = [psp.tile([128, LT], f32, tag=f"f2{i}", bufs=2,
                           name=f"ps_f2{i}") for i in range(NLT_OWN)]
            for mc in range(MC):
                for lt in range(NLT_OWN):
                    nc.tensor.matmul(
                        ps[lt], wti[:, mc, :],
                        gelu_sb[:, mc, lt * LT:(lt + 1) * LT],
                        start=(mc == 0), stop=(mc == MC - 1))
            for lt in range(NLT_OWN):
                gh = sbp.tile([128, LT], f32, tag="gh2", bufs=3, name="gh2")
                nc.scalar.activation(gh, ps[lt], AF.Identity,
                                     bias=gmb2[:, ft:ft + 1],
                                     scale=gate_mlp[:, ft:ft + 1])
                xo = x_own[:, ft, lt * LT:(lt + 1) * LT]
                nc.vector.tensor_tensor(xo, xo, gh, ALU.add)
            # this feature chunk is final; stream it out now so the last
            # DMA is tiny instead of 4.2MB after the last matmul.
            nc.sync.dma_start(out=outr[:, ft, :], in_=x_own[:, ft, :])
    fr_gelu()

    # release persistents
    fr_x_own()
    fr_g2(); fr_g1(); fr_s2(); fr_s1(); fr_tp()
    fr_bv()
    for fr in reversed(bias_frees):
        fr()
    fr_eps(); fr_ones_bf()


_PROGRAM_CACHE = {}


def _get_program():
    if "nc" not in _PROGRAM_CACHE:
        _PROGRAM_CACHE["nc"] = build_program()
    return _PROGRAM_CACHE["nc"]


def _fm(v):
    """[D] vector -> feature-major [128, D//128] (partition p, chunk c)."""
    return np.ascontiguousarray(np.asarray(v, np.float32).reshape(-1, 128).T)


def make_in_maps(x, time_emb, Wqkv, bqkv, Wproj, bproj, W1, b1, W2, b2, Wt, bt,
                 g1, be1, g2, be2):
    # g1/be1/g2/be2 are identity layernorm params in this module; verify and
    # fold them away.
    assert np.allclose(g1, 1.0) and np.allclose(g2, 1.0)
    assert np.allclose(be1, 0.0) and np.allclose(be2, 0.0)

    x = np.asarray(x, np.float32)
    shared = {
        "wqkv": np.asarray(Wqkv, np.float32).astype(BF),
        "bq": _fm(np.asarray(bqkv[:D]) * 0.125),
        "bk": _fm(bqkv[D:2 * D]),
        "bv": np.ascontiguousarray(np.asarray(bqkv[2 * D:], np.float32)[None, :]),
        "wproj": np.asarray(Wproj, np.float32).astype(BF),
        "bproj": _fm(bproj),
        "w1": np.asarray(W1, np.float32).astype(BF),
        "b1": _fm(b1),
        "w2": np.asarray(W2, np.float32).astype(BF),
        "b2": _fm(b2),
        "wt": np.asarray(Wt, np.float32).astype(BF),
        "bt": _fm(bt),
    }
    in_maps = []
    for c in range(NCORES):
        b, half = c // 2, c % 2
        xb = x[b].T  # [D, L] feature-major
        own = slice(half * LOWN, (half + 1) * LOWN)
        oth = slice((1 - half) * LOWN, (2 - half) * LOWN)
        m = dict(shared)
        m["xfm"] = np.ascontiguousarray(xb[:, own])
        m["xoth"] = np.ascontiguousarray(xb[:, oth]).astype(BF)
        m["temb"] = _fm(time_emb[b])
        in_maps.append(m)
    return in_maps


def assemble_output(results):
    outp = np.empty((B, L, D), np.float32)
    for c in range(NCORES):
        b, half = c // 2, c % 2
        outp[b, half * LOWN:(half + 1) * LOWN, :] = results[c]["out_fm"].T
    return outp


def kernel(x, time_emb, Wqkv, bqkv, Wproj, bproj, W1, b1, W2, b2, Wt, bt,
           g1, be1, g2, be2, trace=False, trace_kwargs=None):
    in_maps = make_in_maps(x, time_emb, Wqkv, bqkv, Wproj, bproj, W1, b1,
                           W2, b2, Wt, bt, g1, be1, g2, be2)
    nc = _get_program()
    res = run_bass_kernel_spmd(nc, in_maps, core_ids=list(range(NCORES)),
                               trace=trace, trace_kwargs=trace_kwargs or {})
    kernel.last_results = res
    return assemble_output(res.results)


# revision 35
# speedup vs baseline: 1.2415x; 1.1637x over previous
"""DiT block (adaLN) Trainium2 kernel, 8-core SPMD, no collectives.

Sharding: core c handles batch b = c//2 and query-token half c%2 (1024 q
tokens).  Each core computes K/V for all 2048 tokens of its batch (the
only duplicated work), so cores never communicate.  The host permutes
each core's token columns so its own 1024 tokens come first (softmax is
invariant to key order), and transposes x to feature-major [D, L] so the
device never transposes anything.

On-device layout is feature-major everywhere: activations live as
[128 partitions, d-chunk, tokens].  LayerNorm stats (per-token = free
dim) are computed with ones-vector matmuls on the tensor engine and
broadcast back across partitions with stride-0 DMA.  All GEMM operands
are bf16 (fp32 PSUM accumulation); the residual stream, softmax and LN
statistics stay fp32.

Perf structure (vs the first working version):
- All weights are staged into SBUF with large multi-dim DMAs (one per
  output-feature chunk) instead of one DMA per 128x128 tile; the small
  DMAs serialized on the sync sequencer (~600ns each, ~1150 of them).
- Attention is software-pipelined at key-chunk granularity: the PE does
  QK(g+1) and AV(g-1) while the scalar engine does exp(g), so the PE
  never idles long enough for the HAM clock gate to re-throttle.
- Attention output is written straight into SBUF (partition-offset
  writes per head) instead of bouncing through DRAM.
"""

import os
import sys
from contextlib import ExitStack

os.environ.setdefault("MYCRO_LOCAL_CACHE", "1")
for _p in ("/opt/trn_rl_repo", "/root/.axon_site/_ro/trn_rl_repo"):
    if os.path.isdir(_p) and _p not in sys.path:
        sys.path.insert(0, _p)

import ml_dtypes
import numpy as np

import concourse.bass as bass
import concourse.tile as tile
from concourse import bacc, mybir
from concourse.bass_utils import run_bass_kernel_spmd

B, L, D, H, HD, MLPD = 4, 2048, 1024, 16, 64, 4096
NCORES = 8
LOWN = L // 2          # own query tokens per core
DC = D // 128          # 8 chunks of the model dim
MC = MLPD // 128       # 32 chunks of the mlp dim
LT = 512               # token tile for matmul free dim
NLT_OWN = LOWN // LT   # 2 token tiles (queries)
NKC = L // 128         # 16 key chunks

f32 = mybir.dt.float32
bf16 = mybir.dt.bfloat16
f8 = mybir.dt.float8e4
AF = mybir.ActivationFunctionType
ALU = mybir.AluOpType
BF = ml_dtypes.bfloat16
F8 = ml_dtypes.float8_e4m3
W8SCALE = 64.0  # fp8 MLP weights are pre-scaled by this on the host


def _bcast_rows(nc, pool, row_ap, nrows, ncols, tag, dtype=None, bufs=2):
    """SBUF [nrows, ncols] tile = row_ap ([1, ncols] SBUF) broadcast
    across partitions, on the otherwise-idle GpSimd engine."""
    dtype = dtype or mybir.dt.float32
    out = pool.tile([nrows, ncols], dtype, tag=tag, bufs=bufs, name=tag)
    nc.gpsimd.partition_broadcast(out, row_ap)
    return out


def build_program():
    # Bacc (not plain Bass): its compile() pass legalizes multi-semaphore
    # waits (event semaphores, nop fusion) that walrus can't encode raw.
    nc = bacc.Bacc()

    def _in(name, shape, dtype):
        return nc.declare_dram_parameter(name, shape, dtype, False)[:]

    xfm = _in("xfm", [D, LOWN], f32)
    xoth = _in("xoth", [D, LOWN], bf16)
    temb = _in("temb", [128, DC], f32)
    wqkv = _in("wqkv", [D, 3 * D], bf16)
    bq = _in("bq", [128, DC], f32)     # pre-scaled by 1/8
    bk = _in("bk", [128, DC], f32)
    bv = _in("bv", [1, D], f32)
    wproj = _in("wproj", [D, D], bf16)
    bproj = _in("bproj", [128, DC], f32)
    # fp8 MLP weights, pre-scaled x64 and laid out [p, pair-chunk, 2, cols]
    # for DoubleRow matmuls (contraction pairs (p, i) <-> d = c*256+i*128+p).
    w1 = _in("w1", [128, DC // 2, 2, MLPD], f8)
    b1 = _in("b1", [128, MC], f32)
    w2 = _in("w2", [128, MC // 2, 2, D], f8)
    b2 = _in("b2", [128, DC], f32)
    wt = _in("wt", [D, 6 * D], bf16)
    bt = _in("bt", [128, 48], f32)
    out = nc.declare_dram_parameter("out_fm", [D, LOWN], f32, True)[:]

    with tile.TileContext(nc) as tc:
        _emit_kernel(tc, xfm, xoth, temb, wqkv, bq, bk, bv, wproj, bproj,
                     w1, b1, w2, b2, wt, bt, out)
    nc.finalize()  # runs Bacc.compile(): reg alloc + sync legalization
    return nc


def _emit_kernel(tc, xfm, xoth, temb, wqkv, bq, bk, bv, wproj, bproj, w1, b1,
                 w2, b2, wt, bt, out):
    nc = tc.nc

    # feature-major views of the weight matrices: [128, in-chunk, out-col]
    wt_r = wt.rearrange("(c p) n -> p c n", p=128)
    wqkv_r = wqkv.rearrange("(c p) n -> p c n", p=128)
    wproj_r = wproj.rearrange("(c p) n -> p c n", p=128)

    # ---- persistent constants / host-prepped vectors (freed last) ----
    ones_bf, fr_ones_bf = tc.tile([128, 1], bf16, name="ones_bf")
    nc.vector.memset(ones_bf, 1.0)
    eps_tile, fr_eps = tc.tile([1, 1], f32, name="eps_tile")
    nc.vector.memset(eps_tile, 1e-5)

    bias_sb = {}
    bias_frees = []
    for name, ap, w in (("bq", bq, DC), ("bk", bk, DC), ("bproj", bproj, DC),
                        ("b1", b1, MC), ("b2", b2, DC), ("bt", bt, 48),
                        ("temb", temb, DC)):
        t, fr = tc.tile([128, w], f32, name=f"sb_{name}")
        nc.sync.dma_start(out=t, in_=ap)
        bias_sb[name] = t
        bias_frees.append(fr)
    bv_bc, fr_bv = tc.tile([128, D], f32, name="bv_bc")
    nc.sync.dma_start(
        out=bv_bc,
        in_=bass.AP(tensor=bv.tensor, offset=bv.offset,
                    ap=[[0, 128]] + [list(x) for x in bv.ap[1:]]))

    # modulation vectors (computed in phase 0, consumed later)
    tp, fr_tp = tc.tile([128, 48], f32, name="tp")
    s_msa, fr_s1 = tc.tile([128, DC], f32, name="s_msa")
    s_mlp, fr_s2 = tc.tile([128, DC], f32, name="s_mlp")
    gmbp, fr_g1 = tc.tile([128, DC], f32, name="gmbp")
    gmb2, fr_g2 = tc.tile([128, DC], f32, name="gmb2")
    gm64, fr_g64 = tc.tile([128, DC], f32, name="gm64")
    shift_msa = tp[:, 0:8]
    gate_msa = tp[:, 16:24]
    shift_mlp = tp[:, 24:32]
    gate_mlp = tp[:, 40:48]

    # ---- big persistent activations ----
    x_own, fr_x_own = tc.tile([128, DC, LOWN], f32, name="x_own")
    k_sb, fr_k = tc.tile([128, DC, L], bf16, name="k_sb")
    # v_aug: [token-part, token-chunk, head, 65]; col 64 holds ones so the
    # AV matmul also produces the softmax denominator.
    v_aug, fr_v = tc.tile([128, NKC, H, HD + 1], bf16, name="v_aug")
    q_sb, fr_q = tc.tile([128, DC, LOWN], bf16, name="q_sb")
    xmod, fr_xmod = tc.tile([128, DC, L], bf16, name="xmod")

    xr = xfm.rearrange("(c p) t -> p c t", p=128)
    nc.sync.dma_start(out=x_own, in_=xr)
    # other token half arrives pre-cast to bf16 and lands directly in xmod;
    # LN1 then runs in place on it.
    xor_ = xoth.rearrange("(c p) t -> p c t", p=128)
    nc.sync.dma_start(out=xmod[:, :, LOWN:], in_=xor_)

    # ---- LayerNorm helpers: bf16, in place on a [128, DC, LT] bf16 tile ----
    def ln_stats(sbp, psp, xm_view):
        """Stats over the (pre-modulation) bf16 tile; returns broadcast
        tiles (a_bc=rstd, m_bc=mu*rstd) in bf16."""
        ps_s = psp.tile([1, LT], f32, tag="st_s", bufs=2, name="ps_s")
        ps_q = psp.tile([1, LT], f32, tag="st_q", bufs=2, name="ps_q")
        for dc in range(DC):
            xs = xm_view[:, dc, :]
            nc.tensor.matmul(ps_s, ones_bf, xs,
                             start=(dc == 0), stop=(dc == DC - 1))
            sq = sbp.tile([128, LT], bf16, tag="sq", bufs=2, name="sq")
            nc.vector.tensor_tensor(sq, xs, xs, ALU.mult)
            nc.tensor.matmul(ps_q, ones_bf, sq,
                             start=(dc == 0), stop=(dc == DC - 1))
        mean = sbp.tile([1, LT], f32, tag="ln_mean", bufs=2, name="mean")
        var = sbp.tile([1, LT], f32, tag="ln_var", bufs=2, name="var")
        msq = sbp.tile([1, LT], f32, tag="ln_msq", bufs=2, name="msq")
        nc.vector.tensor_scalar_mul(mean, ps_s, 1.0 / D)
        nc.vector.tensor_scalar_mul(var, ps_q, 1.0 / D)
        nc.vector.tensor_tensor(msq, mean, mean, ALU.mult)
        nc.vector.tensor_tensor(var, var, msq, ALU.subtract)
        # rstd = (var+eps)^-0.5 as Exp(-0.5*Ln(var+eps)): both functions sit
        # in one scalar table set, and it avoids the slow DVE reciprocal.
        lnv = sbp.tile([1, LT], f32, tag="ln_lnv", bufs=2, name="lnv")
        nc.scalar.activation(lnv, var, AF.Ln, bias=eps_tile, scale=1.0)
        rstd = sbp.tile([1, LT], f32, tag="ln_rstd", bufs=2, name="rstd")
        nc.scalar.activation(rstd, lnv, AF.Exp, scale=-0.5)
        mua = sbp.tile([1, LT], f32, tag="ln_mua", bufs=2, name="mua")
        nc.vector.tensor_tensor(mua, mean, rstd, ALU.mult)
        rstd_h = sbp.tile([1, LT], bf16, tag="ln_rsh", bufs=2, name="rstd_h")
        nc.vector.tensor_copy(out=rstd_h, in_=rstd)
        mua_h = sbp.tile([1, LT], bf16, tag="ln_muh", bufs=2, name="mua_h")
        nc.vector.tensor_copy(out=mua_h, in_=mua)
        a_bc = _bcast_rows(nc, sbp, rstd_h, 128, LT, "a_bc", bf16, 4)
        m_bc = _bcast_rows(nc, sbp, mua_h, 128, LT, "m_bc", bf16, 4)
        return a_bc, m_bc

    def ln_apply(sbp, xm_view, a_bc, m_bc, scale_ap, shift_ap, out_view=None,
                 use_scalar=False):
        """out = ((x*a - m) * s + sh, written to out_view (default: in
        place).  With use_scalar the final scale+shift runs on the scalar
        engine (handles fp8 output; DVE does only 2 of the 3 passes)."""
        out_view = xm_view if out_view is None else out_view
        for dc in range(DC):
            t = sbp.tile([128, LT], bf16, tag="ln_t", bufs=2, name="ln_t")
            nc.vector.tensor_tensor(t, xm_view[:, dc, :], a_bc, ALU.mult)
            nc.vector.tensor_tensor(t, t, m_bc, ALU.subtract)
            if use_scalar:
                nc.scalar.activation(out_view[:, dc, :], t, AF.Identity,
                                     bias=shift_ap[:, dc:dc + 1],
                                     scale=scale_ap[:, dc:dc + 1])
            else:
                nc.vector.tensor_scalar(
                    out=out_view[:, dc, :], in0=t,
                    scalar1=scale_ap[:, dc:dc + 1],
                    scalar2=shift_ap[:, dc:dc + 1],
                    op0=ALU.mult, op1=ALU.add)

    # ====== phase 0+1: time modulation vector + LN1, overlapped ======
    # Order: LN1 stats (PE matmuls + DVE) run first; the tp chunks needed
    # by LN1's apply (shift/scale_msa, Wt cols 0:2048) are computed next;
    # then the applies run on DVE while the PE grinds the remaining tp
    # chunks (only needed from proj onward).
    with ExitStack() as ph:
        sbp = ph.enter_context(tc.tile_pool(name="p01_sb", bufs=2))
        psp = ph.enter_context(tc.tile_pool(name="p01_ps", bufs=1,
                                            space="PSUM"))
        sig = sbp.tile([128, DC], f32, tag="sig", bufs=1, name="sig")
        nc.scalar.activation(sig, bias_sb["temb"], AF.Sigmoid)
        silu_bf = sbp.tile([128, DC], bf16, tag="silu", bufs=1, name="silu_bf")
        nc.vector.tensor_tensor(silu_bf, bias_sb["temb"], sig, ALU.mult)

        def tp_chunks(ps_col0, fb_lo, fb_hi, ps_tag):
            ps_t = psp.tile([128, 4 * (fb_hi - fb_lo)], f32, tag=ps_tag,
                            bufs=1, name=ps_tag)
            for fb in range(fb_lo, fb_hi):
                wt_f = sbp.tile([128, DC, 512], bf16, tag="wt", bufs=2,
                                name="wt_f")
                nc.sync.dma_start(out=wt_f,
                                  in_=wt_r[:, :, fb * 512:(fb + 1) * 512])
                for fl in range(4):
                    f = fb * 4 + fl
                    for dc in range(DC):
                        nc.tensor.matmul(
                            ps_t[:, f - ps_col0:f - ps_col0 + 1],
                            wt_f[:, dc, fl * 128:(fl + 1) * 128],
                            silu_bf[:, dc:dc + 1],
                            start=(dc == 0), stop=(dc == DC - 1))
            return ps_t

        # LN1 stats for all 4 token tiles (tp-independent)
        bcs = []
        for t4 in range(4):
            lts = slice(t4 * LT, (t4 + 1) * LT)
            if t4 < NLT_OWN:  # own half: cast the f32 residual into xmod
                nc.vector.tensor_copy(out=xmod[:, :, lts],
                                      in_=x_own[:, :, lts])
            bcs.append(ln_stats(sbp, psp, xmod[:, :, lts]))

        # tp chunks 0:16 -> shift_msa / scale_msa
        ps_a = tp_chunks(0, 0, 4, "tpa")
        nc.vector.tensor_tensor(tp[:, :16], ps_a, bias_sb["bt"][:, :16],
                                ALU.add)
        nc.vector.tensor_scalar_add(s_msa, tp[:, 8:16], 1.0)

        # LN1 applies (DVE) overlap the remaining tp matmuls (PE)
        for t4 in range(4):
            lts = slice(t4 * LT, (t4 + 1) * LT)
            ln_apply(sbp, xmod[:, :, lts], bcs[t4][0], bcs[t4][1],
                     s_msa, shift_msa)

        ps_b = tp_chunks(16, 4, 12, "tpb")
        nc.vector.tensor_tensor(tp[:, 16:], ps_b, bias_sb["bt"][:, 16:],
                                ALU.add)
        nc.vector.tensor_scalar_add(s_mlp, tp[:, 32:40], 1.0)
        nc.vector.tensor_tensor(gmbp, gate_msa, bias_sb["bproj"], ALU.mult)
        nc.vector.tensor_tensor(gmb2, gate_mlp, bias_sb["b2"], ALU.mult)
        nc.vector.tensor_scalar_mul(gm64, gate_mlp, 1.0 / W8SCALE)

    # ================= phase 2: QKV ================
    nc.vector.memset(v_aug[:, :, :, HD:], 1.0)
    with ExitStack() as ph:
        sbp = ph.enter_context(tc.tile_pool(name="p2_sb", bufs=2))
        psp = ph.enter_context(tc.tile_pool(name="p2_ps", bufs=1, space="PSUM"))
        # Q and K: weight-stationary.  ft 0..7 -> q (own tokens only),
        # ft 8..15 -> k (all tokens).
        for ft in range(16):
            is_q = ft < 8
            nlt = NLT_OWN if is_q else L // LT
            wti = sbp.tile([128, DC, 128], bf16, tag="wqk", bufs=3, name="wti")
            nc.sync.dma_start(out=wti,
                              in_=wqkv_r[:, :, ft * 128:(ft + 1) * 128])
            for lt0 in range(0, nlt, 2):
                nl = min(2, nlt - lt0)
                ps = [psp.tile([128, LT], f32, tag=f"qk{i}", bufs=2,
                               name=f"ps_qk{i}") for i in range(nl)]
                for dc in range(DC):
                    for i in range(nl):
                        lt = lt0 + i
                        nc.tensor.matmul(
                            ps[i], wti[:, dc, :],
                            xmod[:, dc, lt * LT:(lt + 1) * LT],
                            start=(dc == 0), stop=(dc == DC - 1))
                for i in range(nl):
                    lt = lt0 + i
                    if is_q:
                        nc.scalar.activation(
                            q_sb[:, ft, lt * LT:(lt + 1) * LT], ps[i],
                            AF.Identity, bias=bias_sb["bq"][:, ft:ft + 1],
                            scale=0.125)
                    else:
                        nc.scalar.activation(
                            k_sb[:, ft - 8, lt * LT:(lt + 1) * LT], ps[i],
                            AF.Identity, bias=bias_sb["bk"][:, ft - 8:ft - 7])

        # V: x-stationary so it lands token-major.
        wv_sb, fr_wv = tc.tile([128, DC, D], bf16, name="wv_sb")
        nc.sync.dma_start(out=wv_sb, in_=wqkv_r[:, :, 2 * D:3 * D])
        for tcn in range(NKC):
            psv = psp.tile([128, 2, LT], f32, tag="v", bufs=2, name="ps_v")
            for dc in range(DC):
                for vs in range(2):
                    nc.tensor.matmul(
                        psv[:, vs, :],
                        xmod[:, dc, tcn * 128:(tcn + 1) * 128],
                        wv_sb[:, dc, vs * LT:(vs + 1) * LT],
                        start=(dc == 0), stop=(dc == DC - 1))
            for vs in range(2):
                nc.vector.tensor_tensor(
                    v_aug[:, tcn, vs * 8:(vs + 1) * 8, :HD],
                    psv[:, vs, :], bv_bc[:, vs * LT:(vs + 1) * LT], ALU.add)
        fr_wv()
    fr_xmod()
    attn_sb, fr_attn = tc.tile([128, DC, LOWN], bf16, name="attn_sb")

    # ================= phase 3: attention ================
    # Software-pipelined per key-chunk g: PE runs QK(g) then AV(g-1) while
    # the scalar engine (the bottleneck: 33.5M exp elements) runs exp(g-1).
    # PSUM budget: qk pair tile 2 banks x2 bufs + two av tiles x2 bufs = 8.
    with ExitStack() as ph:
        sbp = ph.enter_context(tc.tile_pool(name="p3_sb", bufs=2))
        psp = ph.enter_context(tc.tile_pool(name="p3_ps", bufs=1, space="PSUM"))

        def emit_qk_exp(hc, lt, g):
            lts = slice(lt * LT, (lt + 1) * LT)
            ms = slice(g * 128, (g + 1) * 128)
            ps_pair = psp.tile([128, 2, LT], f32, tag="qk", bufs=2,
                               name="ps_pair")
            nc.tensor.matmul(ps_pair[:, 0, :], k_sb[0:64, hc, ms],
                             q_sb[0:64, hc, lts],
                             start=True, stop=True, tile_position=(0, 0))
            nc.tensor.matmul(ps_pair[:, 1, :], k_sb[64:128, hc, ms],
                             q_sb[64:128, hc, lts],
                             start=True, stop=True, tile_position=(64, 0))
            ept = sbp.tile([128, 2, LT], bf16, tag="ept", bufs=3, name="ept")
            nc.scalar.activation(ept, ps_pair, AF.Exp)
            return ept

        for hc in range(H // 2):
            for lt in range(NLT_OWN):
                lts = slice(lt * LT, (lt + 1) * LT)
                ps_av = [psp.tile([HD + 1, LT], f32, tag=f"av{i}", bufs=2,
                                  name=f"ps_av{i}") for i in range(2)]
                prev = emit_qk_exp(hc, lt, 0)
                for g in range(1, NKC):
                    cur = emit_qk_exp(hc, lt, g)
                    for i in range(2):
                        nc.tensor.matmul(ps_av[i], v_aug[:, g - 1, 2 * hc + i, :],
                                         prev[:, i, :],
                                         start=(g == 1), stop=False)
                    prev = cur
                for i in range(2):
                    nc.tensor.matmul(ps_av[i], v_aug[:, NKC - 1, 2 * hc + i, :],
                                     prev[:, i, :], start=False, stop=True)
                for i in range(2):
                    rcp = sbp.tile([1, LT], f32, tag="rcp", bufs=2, name="rcp")
                    nc.vector.reciprocal(out=rcp, in_=ps_av[i][HD:HD + 1, :])
                    rcp_bc = _bcast_rows(nc, sbp, rcp, 64, LT, "rcp_bc")
                    if i == 0:
                        nc.vector.tensor_tensor(
                            attn_sb[0:64, hc, lts],
                            ps_av[0][:HD, :], rcp_bc, ALU.mult)
                    else:
                        # DVE lanes can't shift partitions; bounce head 1
                        # through a small SBUF->SBUF DMA instead of DRAM.
                        at = sbp.tile([64, LT], bf16, tag="at", bufs=2,
                                      name="at")
                        nc.vector.tensor_tensor(at, ps_av[1][:HD, :], rcp_bc,
                                                ALU.mult)
                        nc.sync.dma_start(out=attn_sb[64:128, hc, lts],
                                          in_=at)

    # ================= phase 4: proj + residual ================
    with ExitStack() as ph:
        sbp = ph.enter_context(tc.tile_pool(name="p4_sb", bufs=2))
        psp = ph.enter_context(tc.tile_pool(name="p4_ps", bufs=1, space="PSUM"))
        wp_all = sbp.tile([128, DC, D], bf16, tag="wpj", bufs=1, name="wp_all")
        nc.sync.dma_start(out=wp_all, in_=wproj_r)
        for ft in range(DC):
            ps = [psp.tile([128, LT], f32, tag=f"pj{i}", bufs=2,
                           name=f"ps_pj{i}") for i in range(NLT_OWN)]
            for dc in range(DC):
                for lt in range(NLT_OWN):
                    nc.tensor.matmul(
                        ps[lt], wp_all[:, dc, ft * 128:(ft + 1) * 128],
                        attn_sb[:, dc, lt * LT:(lt + 1) * LT],
                        start=(dc == 0), stop=(dc == DC - 1))
            for lt in range(NLT_OWN):
                gh = sbp.tile([128, LT], f32, tag="gh", bufs=3, name="gh")
                nc.scalar.activation(gh, ps[lt], AF.Identity,
                                     bias=gmbp[:, ft:ft + 1],
                                     scale=gate_msa[:, ft:ft + 1])
                xo = x_own[:, ft, lt * LT:(lt + 1) * LT]
                nc.vector.tensor_tensor(xo, xo, gh, ALU.add)
    fr_attn()
    fr_q()
    fr_v()
    fr_k()

    outr = out.rearrange("(c p) t -> p c t", p=128)

    # ================= phase 5/6: LN2 + MLP (fp8 DoubleRow) ================
    # MLP weights are fp8e4, pre-scaled x64 on the host; 1/64 is folded
    # into the gelu scale (fc1) and the gate scale gm64 (fc2).  DoubleRow
    # contracts 256 rows per matmul: lhsT [128, 2, M], rhs [128, 2, N].
    gelu_sb, fr_gelu = tc.tile([128, MC, LOWN], f8, name="gelu_sb")
    h2mod, fr_h2 = tc.tile([128, DC, LOWN], bf16, name="h2mod")
    h2f8, fr_h2f8 = tc.tile([128, DC, LOWN], f8, name="h2f8")
    with ExitStack() as ph:
        sbp = ph.enter_context(tc.tile_pool(name="p5_sb", bufs=2))
        psp = ph.enter_context(tc.tile_pool(name="p5_ps", bufs=1, space="PSUM"))
        for lt in range(NLT_OWN):
            lts = slice(lt * LT, (lt + 1) * LT)
            nc.vector.tensor_copy(out=h2mod[:, :, lts], in_=x_own[:, :, lts])
            a_bc, m_bc = ln_stats(sbp, psp, h2mod[:, :, lts])
            ln_apply(sbp, h2mod[:, :, lts], a_bc, m_bc, s_mlp, shift_mlp,
                     out_view=h2f8[:, :, lts], use_scalar=True)

    with ExitStack() as ph:
        sbp = ph.enter_context(tc.tile_pool(name="p6_sb", bufs=2))
        psp = ph.enter_context(tc.tile_pool(name="p6_ps", bufs=1, space="PSUM"))
        for ft in range(MC):
            wti = sbp.tile([128, DC // 2, 2, 128], f8, tag="w1t", bufs=4,
                           name="w1t")
            nc.sync.dma_start(out=wti,
                              in_=w1[:, :, :, ft * 128:(ft + 1) * 128])
            ps = [psp.tile([128, LT], f32, tag=f"f1{i}", bufs=2,
                           name=f"ps_f1{i}") for i in range(NLT_OWN)]
            for c in range(DC // 2):
                for lt in range(NLT_OWN):
                    nc.tensor.matmul(
                        ps[lt], wti[:, c, :, :],
                        h2f8[:, 2 * c:2 * c + 2, lt * LT:(lt + 1) * LT],
                        start=(c == 0), stop=(c == DC // 2 - 1),
                        perf_mode=mybir.MatmulPerfMode.DoubleRow)
            for lt in range(NLT_OWN):
                nc.scalar.activation(
                    gelu_sb[:, ft, lt * LT:(lt + 1) * LT], ps[lt], AF.Gelu,
                    bias=bias_sb["b1"][:, ft:ft + 1], scale=1.0 / W8SCALE)
    fr_h2f8()
    fr_h2()

    with ExitStack() as ph:
        sbp = ph.enter_context(tc.tile_pool(name="p7_sb", bufs=2))
        psp = ph.enter_context(tc.tile_pool(name="p7_ps", bufs=1, space="PSUM"))
        for ft in range(DC):
            wti = sbp.tile([128, MC // 2, 2, 128], f8, tag="w2t", bufs=2,
                           name="w2t")
            nc.sync.dma_start(out=wti,
                              in_=w2[:, :, :, ft * 128:(ft + 1) * 128])
            ps = [psp.tile([128, LT], f32, tag=f"f2{i}", bufs=2,
                           name=f"ps_f2{i}") for i in range(NLT_OWN)]
            for m in range(MC // 2):
                for lt in range(NLT_OWN):
                    nc.tensor.matmul(
                        ps[lt], wti[:, m, :, :],
                        gelu_sb[:, 2 * m:2 * m + 2, lt * LT:(lt + 1) * LT],
                        start=(m == 0), stop=(m == MC // 2 - 1),
                        perf_mode=mybir.MatmulPerfMode.DoubleRow)
            for lt in range(NLT_OWN):
                gh = sbp.tile([128, LT], f32, tag="gh2", bufs=3, name="gh2")
                nc.scalar.activation(gh, ps[lt], AF.Identity,
                                     bias=gmb2[:, ft:ft + 1],
                                     scale=gm64[:, ft:ft + 1])
                xo = x_own[:, ft, lt * LT:(lt + 1) * LT]
                nc.vector.tensor_tensor(xo, xo, gh, ALU.add)
            # this feature chunk is final; stream it out now so the last
            # DMA is tiny instead of 4.2MB after the last matmul.
            nc.sync.dma_start(out=outr[:, ft, :], in_=x_own[:, ft, :])
    fr_gelu()

    # release persistents
    fr_x_own()
    fr_g64(); fr_g2(); fr_g1(); fr_s2(); fr_s1(); fr_tp()
    fr_bv()
    for fr in reversed(bias_frees):
        fr()
    fr_eps(); fr_ones_bf()


_PROGRAM_CACHE = {}


def _get_program():
    if "nc" not in _PROGRAM_CACHE:
        _PROGRAM_CACHE["nc"] = build_program()
    return _PROGRAM_CACHE["nc"]


def _fm(v):
    """[D] vector -> feature-major [128, D//128] (partition p, chunk c)."""
    return np.ascontiguousarray(np.asarray(v, np.float32).reshape(-1, 128).T)


def make_in_maps(x, time_emb, Wqkv, bqkv, Wproj, bproj, W1, b1, W2, b2, Wt, bt,
                 g1, be1, g2, be2):
    # g1/be1/g2/be2 are identity layernorm params in this module; verify and
    # fold them away.
    assert np.allclose(g1, 1.0) and np.allclose(g2, 1.0)
    assert np.allclose(be1, 0.0) and np.allclose(be2, 0.0)

    x = np.asarray(x, np.float32)
    shared = {
        "wqkv": np.asarray(Wqkv, np.float32).astype(BF),
        "bq": _fm(np.asarray(bqkv[:D]) * 0.125),
        "bk": _fm(bqkv[D:2 * D]),
        "bv": np.ascontiguousarray(np.asarray(bqkv[2 * D:], np.float32)[None, :]),
        "wproj": np.asarray(Wproj, np.float32).astype(BF),
        "bproj": _fm(bproj),
        # fp8 DoubleRow layout: [p, pair-chunk c, slot i, cols] with
        # d = c*256 + i*128 + p; pre-scaled x64 into e4m3 normal range.
        "w1": np.ascontiguousarray(
            (np.asarray(W1, np.float32) * W8SCALE)
            .reshape(DC // 2, 2, 128, MLPD).transpose(2, 0, 1, 3)).astype(F8),
        "b1": _fm(b1),
        "w2": np.ascontiguousarray(
            (np.asarray(W2, np.float32) * W8SCALE)
            .reshape(MC // 2, 2, 128, D).transpose(2, 0, 1, 3)).astype(F8),
        "b2": _fm(b2),
        "wt": np.asarray(Wt, np.float32).astype(BF),
        "bt": _fm(bt),
    }
    in_maps = []
    for c in range(NCORES):
        b, half = c // 2, c % 2
        xb = x[b].T  # [D, L] feature-major
        own = slice(half * LOWN, (half + 1) * LOWN)
        oth = slice((1 - half) * LOWN, (2 - half) * LOWN)
        m = dict(shared)
        m["xfm"] = np.ascontiguousarray(xb[:, own])
        m["xoth"] = np.ascontiguousarray(xb[:, oth]).astype(BF)
        m["temb"] = _fm(time_emb[b])
        in_maps.append(m)
    return in_maps


def assemble_output(results):
    outp = np.empty((B, L, D), np.float32)
    for c in range(NCORES):
        b, half = c // 2, c % 2
        outp[b, half * LOWN:(half + 1) * LOWN, :] = results[c]["out_fm"].T
    return outp


def kernel(x, time_emb, Wqkv, bqkv, Wproj, bproj, W1, b1, W2, b2, Wt, bt,
           g1, be1, g2, be2, trace=False, trace_kwargs=None):
    in_maps = make_in_maps(x, time_emb, Wqkv, bqkv, Wproj, bproj, W1, b1,
                           W2, b2, Wt, bt, g1, be1, g2, be2)
    nc = _get_program()
    res = run_bass_kernel_spmd(nc, in_maps, core_ids=list(range(NCORES)),
                               trace=trace, trace_kwargs=trace_kwargs or {})
    kernel.last_results = res
    return assemble_output(res.results)


# revision 41
# speedup vs baseline: 1.2445x; 1.0024x over previous
"""DiT block (adaLN) Trainium2 kernel, 8-core SPMD, no collectives.

Sharding: core c handles batch b = c//2 and query-token half c%2 (1024 q
tokens).  Each core computes K/V for all 2048 tokens of its batch (the
only duplicated work), so cores never communicate.  The host permutes
each core's token columns so its own 1024 tokens come first (softmax is
invariant to key order), and transposes x to feature-major [D, L] so the
device never transposes anything.

On-device layout is feature-major everywhere: activations live as
[128 partitions, d-chunk, tokens].  LayerNorm stats (per-token = free
dim) are computed with ones-vector matmuls on the tensor engine and
broadcast back across partitions with stride-0 DMA.  All GEMM operands
are bf16 (fp32 PSUM accumulation); the residual stream, softmax and LN
statistics stay fp32.

Perf structure (vs the first working version):
- All weights are staged into SBUF with large multi-dim DMAs (one per
  output-feature chunk) instead of one DMA per 128x128 tile; the small
  DMAs serialized on the sync sequencer (~600ns each, ~1150 of them).
- Attention is software-pipelined at key-chunk granularity: the PE does
  QK(g+1) and AV(g-1) while the scalar engine does exp(g), so the PE
  never idles long enough for the HAM clock gate to re-throttle.
- Attention output is written straight into SBUF (partition-offset
  writes per head) instead of bouncing through DRAM.
"""

import os
import sys
from contextlib import ExitStack

os.environ.setdefault("MYCRO_LOCAL_CACHE", "1")
for _p in ("/opt/trn_rl_repo", "/root/.axon_site/_ro/trn_rl_repo"):
    if os.path.isdir(_p) and _p not in sys.path:
        sys.path.insert(0, _p)

import ml_dtypes
import numpy as np

import concourse.bass as bass
import concourse.tile as tile
from concourse import bacc, mybir
from concourse.bass_utils import run_bass_kernel_spmd

B, L, D, H, HD, MLPD = 4, 2048, 1024, 16, 64, 4096
NCORES = 8
LOWN = L // 2          # own query tokens per core
DC = D // 128          # 8 chunks of the model dim
MC = MLPD // 128       # 32 chunks of the mlp dim
LT = 512               # token tile for matmul free dim
NLT_OWN = LOWN // LT   # 2 token tiles (queries)
NKC = L // 128         # 16 key chunks

f32 = mybir.dt.float32
bf16 = mybir.dt.bfloat16
f8 = mybir.dt.float8e4
AF = mybir.ActivationFunctionType
ALU = mybir.AluOpType
BF = ml_dtypes.bfloat16
F8 = ml_dtypes.float8_e4m3
W8SCALE = 64.0  # fp8 MLP weights are pre-scaled by this on the host


def _bcast_rows(nc, pool, row_ap, nrows, ncols, tag, dtype=None, bufs=2):
    """SBUF [nrows, ncols] tile = row_ap ([1, ncols] SBUF) broadcast
    across partitions, on the otherwise-idle GpSimd engine."""
    dtype = dtype or mybir.dt.float32
    out = pool.tile([nrows, ncols], dtype, tag=tag, bufs=bufs, name=tag)
    nc.gpsimd.partition_broadcast(out, row_ap)
    return out


def build_program():
    # Bacc (not plain Bass): its compile() pass legalizes multi-semaphore
    # waits (event semaphores, nop fusion) that walrus can't encode raw.
    nc = bacc.Bacc()

    def _in(name, shape, dtype):
        return nc.declare_dram_parameter(name, shape, dtype, False)[:]

    xfm = _in("xfm", [D, LOWN], f32)
    xoth = _in("xoth", [D, LOWN], bf16)
    temb = _in("temb", [128, DC], f32)
    wqkv = _in("wqkv", [D, 3 * D], bf16)
    bq = _in("bq", [128, DC], f32)     # pre-scaled by 1/8
    bk = _in("bk", [128, DC], f32)
    bv = _in("bv", [1, D], f32)
    wproj = _in("wproj", [D, D], bf16)
    bproj = _in("bproj", [128, DC], f32)
    # fp8 MLP weights, pre-scaled x64 and laid out [p, pair-chunk, 2, cols]
    # for DoubleRow matmuls (contraction pairs (p, i) <-> d = c*256+i*128+p).
    w1 = _in("w1", [128, DC // 2, 2, MLPD], f8)
    b1 = _in("b1", [128, MC], f32)
    w2 = _in("w2", [128, MC // 2, 2, D], f8)
    b2 = _in("b2", [128, DC], f32)
    wt = _in("wt", [D, 6 * D], bf16)
    bt = _in("bt", [128, 48], f32)
    out = nc.declare_dram_parameter("out_fm", [D, LOWN], f32, True)[:]

    with tile.TileContext(nc) as tc:
        _emit_kernel(tc, xfm, xoth, temb, wqkv, bq, bk, bv, wproj, bproj,
                     w1, b1, w2, b2, wt, bt, out)
    nc.finalize()  # runs Bacc.compile(): reg alloc + sync legalization
    return nc


def _emit_kernel(tc, xfm, xoth, temb, wqkv, bq, bk, bv, wproj, bproj, w1, b1,
                 w2, b2, wt, bt, out):
    nc = tc.nc

    # feature-major views of the weight matrices: [128, in-chunk, out-col]
    wt_r = wt.rearrange("(c p) n -> p c n", p=128)
    wqkv_r = wqkv.rearrange("(c p) n -> p c n", p=128)
    wproj_r = wproj.rearrange("(c p) n -> p c n", p=128)

    # ---- persistent constants / host-prepped vectors (freed last) ----
    ones_bf, fr_ones_bf = tc.tile([128, 1], bf16, name="ones_bf")
    nc.vector.memset(ones_bf, 1.0)
    eps_tile, fr_eps = tc.tile([1, 1], f32, name="eps_tile")
    nc.vector.memset(eps_tile, 1e-5)

    bias_sb = {}
    bias_frees = []
    for name, ap, w in (("bq", bq, DC), ("bk", bk, DC), ("bproj", bproj, DC),
                        ("b1", b1, MC), ("b2", b2, DC), ("bt", bt, 48),
                        ("temb", temb, DC)):
        t, fr = tc.tile([128, w], f32, name=f"sb_{name}")
        nc.sync.dma_start(out=t, in_=ap)
        bias_sb[name] = t
        bias_frees.append(fr)
    bv_bc, fr_bv = tc.tile([128, D], f32, name="bv_bc")
    nc.sync.dma_start(
        out=bv_bc,
        in_=bass.AP(tensor=bv.tensor, offset=bv.offset,
                    ap=[[0, 128]] + [list(x) for x in bv.ap[1:]]))

    # modulation vectors (computed in phase 0, consumed later)
    tp, fr_tp = tc.tile([128, 48], f32, name="tp")
    s_msa, fr_s1 = tc.tile([128, DC], f32, name="s_msa")
    s_mlp, fr_s2 = tc.tile([128, DC], f32, name="s_mlp")
    gmbp, fr_g1 = tc.tile([128, DC], f32, name="gmbp")
    gmb2, fr_g2 = tc.tile([128, DC], f32, name="gmb2")
    gm64, fr_g64 = tc.tile([128, DC], f32, name="gm64")
    shift_msa = tp[:, 0:8]
    gate_msa = tp[:, 16:24]
    shift_mlp = tp[:, 24:32]
    gate_mlp = tp[:, 40:48]

    # ---- big persistent activations ----
    x_own, fr_x_own = tc.tile([128, DC, LOWN], f32, name="x_own")
    k_sb, fr_k = tc.tile([128, DC, L], bf16, name="k_sb")
    # v_aug: [token-part, token-chunk, head, 65]; col 64 holds ones so the
    # AV matmul also produces the softmax denominator.
    v_aug, fr_v = tc.tile([128, NKC, H, HD + 1], bf16, name="v_aug")
    q_sb, fr_q = tc.tile([128, DC, LOWN], bf16, name="q_sb")
    xmod, fr_xmod = tc.tile([128, DC, L], bf16, name="xmod")

    xr = xfm.rearrange("(c p) t -> p c t", p=128)
    nc.sync.dma_start(out=x_own, in_=xr)
    # other token half arrives pre-cast to bf16 and lands directly in xmod;
    # LN1 then runs in place on it.
    xor_ = xoth.rearrange("(c p) t -> p c t", p=128)
    nc.sync.dma_start(out=xmod[:, :, LOWN:], in_=xor_)

    # ---- LayerNorm helpers: bf16, in place on a [128, DC, LT] bf16 tile ----
    def ln_stats(sbp, psp, xm_view):
        """Stats over the (pre-modulation) bf16 tile; returns broadcast
        tiles (a_bc=rstd, m_bc=mu*rstd) in bf16."""
        ps_s = psp.tile([1, LT], f32, tag="st_s", bufs=2, name="ps_s")
        ps_q = psp.tile([1, LT], f32, tag="st_q", bufs=2, name="ps_q")
        for dc in range(DC):
            xs = xm_view[:, dc, :]
            nc.tensor.matmul(ps_s, ones_bf, xs,
                             start=(dc == 0), stop=(dc == DC - 1))
            sq = sbp.tile([128, LT], bf16, tag="sq", bufs=2, name="sq")
            nc.vector.tensor_tensor(sq, xs, xs, ALU.mult)
            nc.tensor.matmul(ps_q, ones_bf, sq,
                             start=(dc == 0), stop=(dc == DC - 1))
        mean = sbp.tile([1, LT], f32, tag="ln_mean", bufs=2, name="mean")
        var = sbp.tile([1, LT], f32, tag="ln_var", bufs=2, name="var")
        msq = sbp.tile([1, LT], f32, tag="ln_msq", bufs=2, name="msq")
        nc.vector.tensor_scalar_mul(mean, ps_s, 1.0 / D)
        nc.vector.tensor_scalar_mul(var, ps_q, 1.0 / D)
        nc.vector.tensor_tensor(msq, mean, mean, ALU.mult)
        nc.vector.tensor_tensor(var, var, msq, ALU.subtract)
        # rstd = (var+eps)^-0.5 as Exp(-0.5*Ln(var+eps)): both functions sit
        # in one scalar table set, and it avoids the slow DVE reciprocal.
        lnv = sbp.tile([1, LT], f32, tag="ln_lnv", bufs=2, name="lnv")
        nc.scalar.activation(lnv, var, AF.Ln, bias=eps_tile, scale=1.0)
        rstd = sbp.tile([1, LT], f32, tag="ln_rstd", bufs=2, name="rstd")
        nc.scalar.activation(rstd, lnv, AF.Exp, scale=-0.5)
        mua = sbp.tile([1, LT], f32, tag="ln_mua", bufs=2, name="mua")
        nc.vector.tensor_tensor(mua, mean, rstd, ALU.mult)
        rstd_h = sbp.tile([1, LT], bf16, tag="ln_rsh", bufs=2, name="rstd_h")
        nc.vector.tensor_copy(out=rstd_h, in_=rstd)
        mua_h = sbp.tile([1, LT], bf16, tag="ln_muh", bufs=2, name="mua_h")
        nc.vector.tensor_copy(out=mua_h, in_=mua)
        a_bc = _bcast_rows(nc, sbp, rstd_h, 128, LT, "a_bc", bf16, 4)
        m_bc = _bcast_rows(nc, sbp, mua_h, 128, LT, "m_bc", bf16, 4)
        return a_bc, m_bc

    def ln_apply(sbp, xm_view, a_bc, m_bc, scale_ap, shift_ap, out_view=None,
                 use_scalar=False):
        """out = ((x*a - m) * s + sh, written to out_view (default: in
        place).  With use_scalar the final scale+shift runs on the scalar
        engine (handles fp8 output; DVE does only 2 of the 3 passes)."""
        out_view = xm_view if out_view is None else out_view
        for dc in range(DC):
            t = sbp.tile([128, LT], bf16, tag="ln_t", bufs=2, name="ln_t")
            nc.vector.tensor_tensor(t, xm_view[:, dc, :], a_bc, ALU.mult)
            nc.vector.tensor_tensor(t, t, m_bc, ALU.subtract)
            if use_scalar:
                nc.scalar.activation(out_view[:, dc, :], t, AF.Identity,
                                     bias=shift_ap[:, dc:dc + 1],
                                     scale=scale_ap[:, dc:dc + 1])
            else:
                nc.vector.tensor_scalar(
                    out=out_view[:, dc, :], in0=t,
                    scalar1=scale_ap[:, dc:dc + 1],
                    scalar2=shift_ap[:, dc:dc + 1],
                    op0=ALU.mult, op1=ALU.add)

    # ====== phase 0+1: time modulation vector + LN1, overlapped ======
    # Order: LN1 stats (PE matmuls + DVE) run first; the tp chunks needed
    # by LN1's apply (shift/scale_msa, Wt cols 0:2048) are computed next;
    # then the applies run on DVE while the PE grinds the remaining tp
    # chunks (only needed from proj onward).
    with ExitStack() as ph:
        sbp = ph.enter_context(tc.tile_pool(name="p01_sb", bufs=2))
        psp = ph.enter_context(tc.tile_pool(name="p01_ps", bufs=1,
                                            space="PSUM"))
        sig = sbp.tile([128, DC], f32, tag="sig", bufs=1, name="sig")
        nc.scalar.activation(sig, bias_sb["temb"], AF.Sigmoid)
        silu_bf = sbp.tile([128, DC], bf16, tag="silu", bufs=1, name="silu_bf")
        nc.vector.tensor_tensor(silu_bf, bias_sb["temb"], sig, ALU.mult)

        def tp_chunks(ps_col0, fb_lo, fb_hi, ps_tag):
            ps_t = psp.tile([128, 4 * (fb_hi - fb_lo)], f32, tag=ps_tag,
                            bufs=1, name=ps_tag)
            for fb in range(fb_lo, fb_hi):
                wt_f = sbp.tile([128, DC, 512], bf16, tag="wt", bufs=2,
                                name="wt_f")
                nc.sync.dma_start(out=wt_f,
                                  in_=wt_r[:, :, fb * 512:(fb + 1) * 512])
                for fl in range(4):
                    f = fb * 4 + fl
                    for dc in range(DC):
                        nc.tensor.matmul(
                            ps_t[:, f - ps_col0:f - ps_col0 + 1],
                            wt_f[:, dc, fl * 128:(fl + 1) * 128],
                            silu_bf[:, dc:dc + 1],
                            start=(dc == 0), stop=(dc == DC - 1))
            return ps_t

        # LN1 stats for all 4 token tiles (tp-independent)
        bcs = []
        for t4 in range(4):
            lts = slice(t4 * LT, (t4 + 1) * LT)
            if t4 < NLT_OWN:  # own half: cast the f32 residual into xmod
                nc.vector.tensor_copy(out=xmod[:, :, lts],
                                      in_=x_own[:, :, lts])
            bcs.append(ln_stats(sbp, psp, xmod[:, :, lts]))

        # tp chunks 0:16 -> shift_msa / scale_msa
        ps_a = tp_chunks(0, 0, 4, "tpa")
        nc.vector.tensor_tensor(tp[:, :16], ps_a, bias_sb["bt"][:, :16],
                                ALU.add)
        nc.vector.tensor_scalar_add(s_msa, tp[:, 8:16], 1.0)

        # LN1 applies (DVE) overlap the remaining tp matmuls (PE)
        for t4 in range(4):
            lts = slice(t4 * LT, (t4 + 1) * LT)
            ln_apply(sbp, xmod[:, :, lts], bcs[t4][0], bcs[t4][1],
                     s_msa, shift_msa, use_scalar=True)

        ps_b = tp_chunks(16, 4, 12, "tpb")
        nc.vector.tensor_tensor(tp[:, 16:], ps_b, bias_sb["bt"][:, 16:],
                                ALU.add)
        nc.vector.tensor_scalar_add(s_mlp, tp[:, 32:40], 1.0)
        nc.vector.tensor_tensor(gmbp, gate_msa, bias_sb["bproj"], ALU.mult)
        nc.vector.tensor_tensor(gmb2, gate_mlp, bias_sb["b2"], ALU.mult)
        nc.vector.tensor_scalar_mul(gm64, gate_mlp, 1.0 / W8SCALE)

    # ================= phase 2: QKV ================
    nc.vector.memset(v_aug[:, :, :, HD:], 1.0)
    with ExitStack() as ph:
        sbp = ph.enter_context(tc.tile_pool(name="p2_sb", bufs=2))
        psp = ph.enter_context(tc.tile_pool(name="p2_ps", bufs=1, space="PSUM"))
        # Q and K: weight-stationary.  ft 0..7 -> q (own tokens only),
        # ft 8..15 -> k (all tokens).
        for ft in range(16):
            is_q = ft < 8
            nlt = NLT_OWN if is_q else L // LT
            wti = sbp.tile([128, DC, 128], bf16, tag="wqk", bufs=3, name="wti")
            nc.sync.dma_start(out=wti,
                              in_=wqkv_r[:, :, ft * 128:(ft + 1) * 128])
            for lt0 in range(0, nlt, 2):
                nl = min(2, nlt - lt0)
                ps = [psp.tile([128, LT], f32, tag=f"qk{i}", bufs=2,
                               name=f"ps_qk{i}") for i in range(nl)]
                for dc in range(DC):
                    for i in range(nl):
                        lt = lt0 + i
                        nc.tensor.matmul(
                            ps[i], wti[:, dc, :],
                            xmod[:, dc, lt * LT:(lt + 1) * LT],
                            start=(dc == 0), stop=(dc == DC - 1))
                for i in range(nl):
                    lt = lt0 + i
                    if is_q:
                        nc.scalar.activation(
                            q_sb[:, ft, lt * LT:(lt + 1) * LT], ps[i],
                            AF.Identity, bias=bias_sb["bq"][:, ft:ft + 1],
                            scale=0.125)
                    else:
                        nc.scalar.activation(
                            k_sb[:, ft - 8, lt * LT:(lt + 1) * LT], ps[i],
                            AF.Identity, bias=bias_sb["bk"][:, ft - 8:ft - 7])

        # V: x-stationary so it lands token-major.
        wv_sb, fr_wv = tc.tile([128, DC, D], bf16, name="wv_sb")
        nc.sync.dma_start(out=wv_sb, in_=wqkv_r[:, :, 2 * D:3 * D])
        for tcn in range(NKC):
            psv = psp.tile([128, 2, LT], f32, tag="v", bufs=2, name="ps_v")
            for dc in range(DC):
                for vs in range(2):
                    nc.tensor.matmul(
                        psv[:, vs, :],
                        xmod[:, dc, tcn * 128:(tcn + 1) * 128],
                        wv_sb[:, dc, vs * LT:(vs + 1) * LT],
                        start=(dc == 0), stop=(dc == DC - 1))
            for vs in range(2):
                nc.vector.tensor_tensor(
                    v_aug[:, tcn, vs * 8:(vs + 1) * 8, :HD],
                    psv[:, vs, :], bv_bc[:, vs * LT:(vs + 1) * LT], ALU.add)
        fr_wv()
    fr_xmod()
    attn_sb, fr_attn = tc.tile([128, DC, LOWN], bf16, name="attn_sb")

    # ================= phase 3: attention ================
    # Software-pipelined per key-chunk g: PE runs QK(g) then AV(g-1) while
    # the scalar engine (the bottleneck: 33.5M exp elements) runs exp(g-1).
    # PSUM budget: qk pair tile 2 banks x2 bufs + two av tiles x2 bufs = 8.
    with ExitStack() as ph:
        sbp = ph.enter_context(tc.tile_pool(name="p3_sb", bufs=2))
        psp = ph.enter_context(tc.tile_pool(name="p3_ps", bufs=1, space="PSUM"))

        def emit_qk_exp(hc, lt, g):
            lts = slice(lt * LT, (lt + 1) * LT)
            ms = slice(g * 128, (g + 1) * 128)
            ps_pair = psp.tile([128, 2, LT], f32, tag="qk", bufs=2,
                               name="ps_pair")
            nc.tensor.matmul(ps_pair[:, 0, :], k_sb[0:64, hc, ms],
                             q_sb[0:64, hc, lts],
                             start=True, stop=True, tile_position=(0, 0))
            nc.tensor.matmul(ps_pair[:, 1, :], k_sb[64:128, hc, ms],
                             q_sb[64:128, hc, lts],
                             start=True, stop=True, tile_position=(64, 0))
            ept = sbp.tile([128, 2, LT], bf16, tag="ept", bufs=3, name="ept")
            nc.scalar.activation(ept, ps_pair, AF.Exp)
            return ept

        for hc in range(H // 2):
            for lt in range(NLT_OWN):
                lts = slice(lt * LT, (lt + 1) * LT)
                ps_av = [psp.tile([HD + 1, LT], f32, tag=f"av{i}", bufs=2,
                                  name=f"ps_av{i}") for i in range(2)]
                prev = emit_qk_exp(hc, lt, 0)
                for g in range(1, NKC):
                    cur = emit_qk_exp(hc, lt, g)
                    for i in range(2):
                        nc.tensor.matmul(ps_av[i], v_aug[:, g - 1, 2 * hc + i, :],
                                         prev[:, i, :],
                                         start=(g == 1), stop=False)
                    prev = cur
                for i in range(2):
                    nc.tensor.matmul(ps_av[i], v_aug[:, NKC - 1, 2 * hc + i, :],
                                     prev[:, i, :], start=False, stop=True)
                for i in range(2):
                    rcp = sbp.tile([1, LT], f32, tag="rcp", bufs=2, name="rcp")
                    nc.vector.reciprocal(out=rcp, in_=ps_av[i][HD:HD + 1, :])
                    rcp_bc = _bcast_rows(nc, sbp, rcp, 64, LT, "rcp_bc")
                    if i == 0:
                        nc.vector.tensor_tensor(
                            attn_sb[0:64, hc, lts],
                            ps_av[0][:HD, :], rcp_bc, ALU.mult)
                    else:
                        # DVE lanes can't shift partitions; bounce head 1
                        # through a small SBUF->SBUF DMA instead of DRAM.
                        at = sbp.tile([64, LT], bf16, tag="at", bufs=2,
                                      name="at")
                        nc.vector.tensor_tensor(at, ps_av[1][:HD, :], rcp_bc,
                                                ALU.mult)
                        nc.sync.dma_start(out=attn_sb[64:128, hc, lts],
                                          in_=at)

    # ================= phase 4: proj + residual ================
    with ExitStack() as ph:
        sbp = ph.enter_context(tc.tile_pool(name="p4_sb", bufs=2))
        psp = ph.enter_context(tc.tile_pool(name="p4_ps", bufs=1, space="PSUM"))
        wp_all = sbp.tile([128, DC, D], bf16, tag="wpj", bufs=1, name="wp_all")
        nc.sync.dma_start(out=wp_all, in_=wproj_r)
        for ft in range(DC):
            ps = [psp.tile([128, LT], f32, tag=f"pj{i}", bufs=2,
                           name=f"ps_pj{i}") for i in range(NLT_OWN)]
            for dc in range(DC):
                for lt in range(NLT_OWN):
                    nc.tensor.matmul(
                        ps[lt], wp_all[:, dc, ft * 128:(ft + 1) * 128],
                        attn_sb[:, dc, lt * LT:(lt + 1) * LT],
                        start=(dc == 0), stop=(dc == DC - 1))
            for lt in range(NLT_OWN):
                gh = sbp.tile([128, LT], f32, tag="gh", bufs=3, name="gh")
                nc.scalar.activation(gh, ps[lt], AF.Identity,
                                     bias=gmbp[:, ft:ft + 1],
                                     scale=gate_msa[:, ft:ft + 1])
                xo = x_own[:, ft, lt * LT:(lt + 1) * LT]
                nc.vector.tensor_tensor(xo, xo, gh, ALU.add)
    fr_attn()
    fr_q()
    fr_v()
    fr_k()

    outr = out.rearrange("(c p) t -> p c t", p=128)

    # ================= phase 5/6: LN2 + MLP (fp8 DoubleRow) ================
    # MLP weights are fp8e4, pre-scaled x64 on the host; 1/64 is folded
    # into the gelu scale (fc1) and the gate scale gm64 (fc2).  DoubleRow
    # contracts 256 rows per matmul: lhsT [128, 2, M], rhs [128, 2, N].
    gelu_sb, fr_gelu = tc.tile([128, MC, LOWN], f8, name="gelu_sb")
    h2mod, fr_h2 = tc.tile([128, DC, LOWN], bf16, name="h2mod")
    h2f8, fr_h2f8 = tc.tile([128, DC, LOWN], f8, name="h2f8")
    with ExitStack() as ph:
        sbp = ph.enter_context(tc.tile_pool(name="p5_sb", bufs=2))
        psp = ph.enter_context(tc.tile_pool(name="p5_ps", bufs=1, space="PSUM"))
        for lt in range(NLT_OWN):
            lts = slice(lt * LT, (lt + 1) * LT)
            nc.vector.tensor_copy(out=h2mod[:, :, lts], in_=x_own[:, :, lts])
            a_bc, m_bc = ln_stats(sbp, psp, h2mod[:, :, lts])
            ln_apply(sbp, h2mod[:, :, lts], a_bc, m_bc, s_mlp, shift_mlp,
                     out_view=h2f8[:, :, lts], use_scalar=True)

    with ExitStack() as ph:
        sbp = ph.enter_context(tc.tile_pool(name="p6_sb", bufs=2))
        psp = ph.enter_context(tc.tile_pool(name="p6_ps", bufs=1, space="PSUM"))
        for ft in range(MC):
            wti = sbp.tile([128, DC // 2, 2, 128], f8, tag="w1t", bufs=4,
                           name="w1t")
            nc.sync.dma_start(out=wti,
                              in_=w1[:, :, :, ft * 128:(ft + 1) * 128])
            ps = [psp.tile([128, LT], f32, tag=f"f1{i}", bufs=2,
                           name=f"ps_f1{i}") for i in range(NLT_OWN)]
            for c in range(DC // 2):
                for lt in range(NLT_OWN):
                    nc.tensor.matmul(
                        ps[lt], wti[:, c, :, :],
                        h2f8[:, 2 * c:2 * c + 2, lt * LT:(lt + 1) * LT],
                        start=(c == 0), stop=(c == DC // 2 - 1),
                        perf_mode=mybir.MatmulPerfMode.DoubleRow)
            for lt in range(NLT_OWN):
                nc.scalar.activation(
                    gelu_sb[:, ft, lt * LT:(lt + 1) * LT], ps[lt], AF.Gelu,
                    bias=bias_sb["b1"][:, ft:ft + 1], scale=1.0 / W8SCALE)
    fr_h2f8()
    fr_h2()

    with ExitStack() as ph:
        sbp = ph.enter_context(tc.tile_pool(name="p7_sb", bufs=2))
        psp = ph.enter_context(tc.tile_pool(name="p7_ps", bufs=1, space="PSUM"))
        for ft in range(DC):
            wti = sbp.tile([128, MC // 2, 2, 128], f8, tag="w2t", bufs=2,
                           name="w2t")
            nc.sync.dma_start(out=wti,
                              in_=w2[:, :, :, ft * 128:(ft + 1) * 128])
            ps = [psp.tile([128, LT], f32, tag=f"f2{i}", bufs=2,
                           name=f"ps_f2{i}") for i in range(NLT_OWN)]
            for m in range(MC // 2):
                for lt in range(NLT_OWN):
                    nc.tensor.matmul(
                        ps[lt], wti[:, m, :, :],
                        gelu_sb[:, 2 * m:2 * m + 2, lt * LT:(lt + 1) * LT],
                        start=(m == 0), stop=(m == MC // 2 - 1),
                        perf_mode=mybir.MatmulPerfMode.DoubleRow)
            for lt in range(NLT_OWN):
                gh = sbp.tile([128, LT], f32, tag="gh2", bufs=3, name="gh2")
                nc.scalar.activation(gh, ps[lt], AF.Identity,
                                     bias=gmb2[:, ft:ft + 1],
                                     scale=gm64[:, ft:ft + 1])
                xo = x_own[:, ft, lt * LT:(lt + 1) * LT]
                nc.vector.tensor_tensor(xo, xo, gh, ALU.add)
            # this feature chunk is final; stream it out now so the last
            # DMA is tiny instead of 4.2MB after the last matmul.
            nc.sync.dma_start(out=outr[:, ft, :], in_=x_own[:, ft, :])
    fr_gelu()

    # release persistents
    fr_x_own()
    fr_g64(); fr_g2(); fr_g1(); fr_s2(); fr_s1(); fr_tp()
    fr_bv()
    for fr in reversed(bias_frees):
        fr()
    fr_eps(); fr_ones_bf()


_PROGRAM_CACHE = {}


def _get_program():
    if "nc" not in _PROGRAM_CACHE:
        _PROGRAM_CACHE["nc"] = build_program()
    return _PROGRAM_CACHE["nc"]


def _fm(v):
    """[D] vector -> feature-major [128, D//128] (partition p, chunk c)."""
    return np.ascontiguousarray(np.asarray(v, np.float32).reshape(-1, 128).T)


def make_in_maps(x, time_emb, Wqkv, bqkv, Wproj, bproj, W1, b1, W2, b2, Wt, bt,
                 g1, be1, g2, be2):
    # g1/be1/g2/be2 are identity layernorm params in this module; verify and
    # fold them away.
    assert np.allclose(g1, 1.0) and np.allclose(g2, 1.0)
    assert np.allclose(be1, 0.0) and np.allclose(be2, 0.0)

    x = np.asarray(x, np.float32)
    shared = {
        "wqkv": np.asarray(Wqkv, np.float32).astype(BF),
        "bq": _fm(np.asarray(bqkv[:D]) * 0.125),
        "bk": _fm(bqkv[D:2 * D]),
        "bv": np.ascontiguousarray(np.asarray(bqkv[2 * D:], np.float32)[None, :]),
        "wproj": np.asarray(Wproj, np.float32).astype(BF),
        "bproj": _fm(bproj),
        # fp8 DoubleRow layout: [p, pair-chunk c, slot i, cols] with
        # d = c*256 + i*128 + p; pre-scaled x64 into e4m3 normal range.
        "w1": np.ascontiguousarray(
            (np.asarray(W1, np.float32) * W8SCALE)
            .reshape(DC // 2, 2, 128, MLPD).transpose(2, 0, 1, 3)).astype(F8),
        "b1": _fm(b1),
        "w2": np.ascontiguousarray(
            (np.asarray(W2, np.float32) * W8SCALE)
            .reshape(MC // 2, 2, 128, D).transpose(2, 0, 1, 3)).astype(F8),
        "b2": _fm(b2),
        "wt": np.asarray(Wt, np.float32).astype(BF),
        "bt": _fm(bt),
    }
    in_maps = []
    for c in range(NCORES):
        b, half = c // 2, c % 2
        xb = x[b].T  # [D, L] feature-major
        own = slice(half * LOWN, (half + 1) * LOWN)
        oth = slice((1 - half) * LOWN, (2 - half) * LOWN)
        m = dict(shared)
        m["xfm"] = np.ascontiguousarray(xb[:, own])
        m["xoth"] = np.ascontiguousarray(xb[:, oth]).astype(BF)
        m["temb"] = _fm(time_emb[b])
        in_maps.append(m)
    return in_maps


def assemble_output(results):
    outp = np.empty((B, L, D), np.float32)
    for c in range(NCORES):
        b, half = c // 2, c % 2
        outp[b, half * LOWN:(half + 1) * LOWN, :] = results[c]["out_fm"].T
    return outp


def kernel(x, time_emb, Wqkv, bqkv, Wproj, bproj, W1, b1, W2, b2, Wt, bt,
           g1, be1, g2, be2, trace=False, trace_kwargs=None):
    in_maps = make_in_maps(x, time_emb, Wqkv, bqkv, Wproj, bproj, W1, b1,
                           W2, b2, Wt, bt, g1, be1, g2, be2)
    nc = _get_program()
    res = run_bass_kernel_spmd(nc, in_maps, core_ids=list(range(NCORES)),
                               trace=trace, trace_kwargs=trace_kwargs or {})
    kernel.last_results = res
    return assemble_output(res.results)
